# revision 13
# baseline (speedup 1.0000x reference)
"""Trainium2 Bass kernel for nn_CascadeTransformerMM (4-layer ternary-GLU cascade).

Math (per layer, per token row):
  h   = rms_scale * x * rsqrt(mean(x^2) + 1e-6)
  s   = clip(127/(max|h| + 1e-5), 1e-3, 1e3);  q = round(s*h)      (ints in [-127,127])
  Wt  = clip(round(W * 127/(max|W| + 1e-5)), -1, 1)                 (ternary {-1,0,1})
  u   = (q @ Wg_t)/s ; v = (q @ Wu_t)/s ; g = silu(u)*v
  s2  = clip(127/(max|g| + 1e-5), 1e-3, 1e3); gq = round(s2*g)
  x  += (gq @ Wd_t)/s2

Distribution: pure data-parallel over the batch dim (8 batches -> 8 cores),
weights replicated per core. Per-matrix |W|max is computed cooperatively:
each core reduces a 1/8 row-slice, then a tiny AllReduce(max) shares the 12
scalars (layer 0's three matrices allreduced first so its ternarize can
start early). All matmuls run on the PE array with bf16 activations
(integers <= 127, exact) x fp8 ternary weights (exact), fp32 PSUM
accumulation -> the heavy compute is bit-exact integer arithmetic.

Schedule: per-token-tile software pipeline with one-tile lookahead --
q/qT production for tile i+2 and down-proj for tile i-1 are emitted
around the up-proj matmul block of tile i, so the serial per-tile tail
(s2 chain -> gq round -> gqT transpose) hides under the next tile's
matmuls and the PE stream stays dense (keeps the HAM clock-gate warm).
Weights live in per-column-chunk SBUF tiles so layer-0 ternarize and
layer-boundary refills unblock the matmul stream progressively.
Ternarize runs pass 1 (scale+round-to-int32) on DVE and pass 2 (clip to
{-1,0,1} as fp8) on the otherwise-idle GPSIMD (alternating with ACT
Sign for layer 0, where latency matters)."""

import os
import sys

for _p in ("/opt/trn_rl_repo", "/root/.axon_site/_ro/trn_rl_repo"):
    if os.path.isdir(_p) and _p not in sys.path:
        sys.path.insert(0, _p)

import numpy as np
from contextlib import ExitStack

import concourse.bass as bass
import concourse.bass_isa as bass_isa
import concourse.mybir as mybir
import concourse.tile as tile
from concourse.bass_utils import run_bass_kernel_spmd

dt = mybir.dt
AF = mybir.ActivationFunctionType
ALU = mybir.AluOpType

MAGIC = float(1.5 * 2**23)  # fp32 round-to-nearest-even magic constant
D = 1024
F = 4096
L = 4
NCORES = 8
TOK = 1024  # tokens per core (one batch of S=1024)

NDK = D // 128   # 8 contraction tiles for up-proj
NFT = F // 128   # 32 contraction tiles for down-proj
NFC = F // 512   # 8 free-dim chunks for up-proj
NCH = F // 1024  # 4 column-chunks in the repacked wg/wu layout
NTT = TOK // 128  # 8 token tiles


def _split_excess_waits(nc, max_waits: int = 1) -> int:
    """walrus in this container rejects >1 sync-wait per instruction; split
    extras into standalone event-semaphore waits on the same engine (same-
    engine program order makes this semantically identical)."""
    n = 0
    for func in nc.m.functions:
        for block in func.blocks:
            changed = False
            out = []
            for inst in block.instructions:
                si = getattr(inst, "sync_info", None)
                if si is not None and si.on_wait and len(si.on_wait) > max_waits:
                    waits = list(si.on_wait)
                    for j, w in enumerate(waits[max_waits:]):
                        out.append(
                            mybir.InstEventSemaphore(
                                name=f"{inst.name}-xw{j}",
                                engine=inst.engine,
                                ins=[],
                                outs=[],
                                sync_info=mybir.SyncInfo(on_wait=[w], on_update=[]),
                            )
                        )
                        n += 1
                    inst.sync_info = mybir.SyncInfo(
                        on_wait=waits[:max_waits], on_update=list(si.on_update)
                    )
                    changed = True
                out.append(inst)
            if changed:
                block.instructions = out
    return n


def build(is_ones: bool = True, n_cores: int = NCORES, n_tok_tiles: int = NTT,
          n_layers: int = L) -> bass.Bass:
    nc = bass.Bass(num_devices=n_cores)
    tok = n_tok_tiles * 128

    x_ext = nc.declare_dram_parameter("x", [tok, D], dt.float32, isOutput=False)
    rs_ext = nc.declare_dram_parameter("rs", [n_layers, D], dt.float32, isOutput=False)
    # wg/wu repacked host-side to [L, F//1024, NDK, 128, 1024] so every
    # [128, 1024] weight tile is one contiguous 512 KB DMA; wd's row-slabs
    # are naturally contiguous.
    wg_ext = nc.declare_dram_parameter("wg", [n_layers, NCH, NDK, 128, 1024], dt.float32, isOutput=False)
    wu_ext = nc.declare_dram_parameter("wu", [n_layers, NCH, NDK, 128, 1024], dt.float32, isOutput=False)
    wd_ext = nc.declare_dram_parameter("wd", [n_layers, F, D], dt.float32, isOutput=False)
    # per-core row-slices of each matrix for the cooperative |W|max,
    # as [L, nun, 128, 1024] contiguous units
    nun = (D // n_cores) * F // (128 * 1024)
    slg_ext = nc.declare_dram_parameter("slg", [n_layers, nun, 128, 1024], dt.float32, isOutput=False)
    slu_ext = nc.declare_dram_parameter("slu", [n_layers, nun, 128, 1024], dt.float32, isOutput=False)
    sld_ext = nc.declare_dram_parameter("sld", [n_layers, nun, 128, 1024], dt.float32, isOutput=False)
    out_ext = nc.declare_dram_parameter("out", [tok, D], dt.float32, isOutput=True)

    mx0_loc = nc.dram_tensor("mx0_loc", [1, 16], dt.float32)
    mx0_glob = nc.dram_tensor("mx0_glob", [1, 16], dt.float32)
    mx1_loc = nc.dram_tensor("mx1_loc", [1, 16], dt.float32)
    mx1_glob = nc.dram_tensor("mx1_glob", [1, 16], dt.float32)

    with tile.TileContext(nc) as tc, ExitStack() as ctx:
        P = ctx.enter_context
        wch = P(tc.tile_pool(name="wch", bufs=2 * NCH))   # per-ch wg/wu fp8 tiles
        wdp = P(tc.tile_pool(name="wdp", bufs=1))         # wd fp8 tile
        wstream = P(tc.tile_pool(name="wstream", bufs=2))
        wi32 = P(tc.tile_pool(name="wi32", bufs=2))
        xpool = P(tc.tile_pool(name="x1", bufs=2))
        xrpool = P(tc.tile_pool(name="xr", bufs=2))
        t1pool = P(tc.tile_pool(name="t1", bufs=1))
        s4k = P(tc.tile_pool(name="s4k", bufs=2))         # xnew scratch
        qpool = P(tc.tile_pool(name="q", bufs=2))
        qtpool = P(tc.tile_pool(name="qt", bufs=2))
        silupool = P(tc.tile_pool(name="silu", bufs=2))   # [128,512] silu chunks
        scrpool = P(tc.tile_pool(name="scr", bufs=1))     # square scratch
        gpool = P(tc.tile_pool(name="g", bufs=1))
        gqpool = P(tc.tile_pool(name="gq", bufs=1))
        gqtpool = P(tc.tile_pool(name="gqt", bufs=2))
        w8s = P(tc.tile_pool(name="w8s", bufs=1))
        batch = P(tc.tile_pool(name="batch", bufs=2))     # [128, ntt] per-layer stats
        btmp = P(tc.tile_pool(name="btmp", bufs=1))       # stats chain temps
        sc = P(tc.tile_pool(name="sc", bufs=4))           # [128, small] scalars
        xdrpool = P(tc.tile_pool(name="xdr", bufs=1))
        const = P(tc.tile_pool(name="const", bufs=1))
        bcpool = P(tc.tile_pool(name="bc", bufs=2))       # rms_scale broadcast (general path)
        dram = P(tc.tile_pool(name="dram", bufs=2, space="DRAM"))
        ps_up = P(tc.tile_pool(name="psup", bufs=4, space="PSUM"))
        ps_dn = P(tc.tile_pool(name="psdn", bufs=4, space="PSUM"))

        # ---------- constants ----------
        mag = const.tile([128, 1], dt.float32, tag="mag")
        nc.gpsimd.memset(mag[:], MAGIC)
        ones1 = const.tile([1, 128], dt.float32, tag="ones1")
        nc.gpsimd.memset(ones1[:], 1.0)

        def pe_broadcast(dst, src_row, n):
            """broadcast src_row [1, n] to dst [128, n] via PE outer product."""
            for h in range(0, n, 512):
                w = min(512, n - h)
                bc_ps = ps_up.tile([128, 512], dt.float32, tag="ups")
                nc.tensor.matmul(bc_ps[:, 0:w], ones1[:], src_row[:, h:h + w],
                                 start=True, stop=True)
                nc.scalar.activation(dst[:, h:h + w], bc_ps[:, 0:w], AF.Copy)

        # ---------- cooperative per-matrix |W|max ----------
        # Layer 0's three matrices are reduced + allreduced first so its
        # ternarize can start while layers 1-3 slices still stream.
        wmax_cols = const.tile([128, 16], dt.float32, tag="wmaxc")
        nc.gpsimd.memset(wmax_cols[:], 0.0)

        def slab_max(l, ext, idx, eng):
            part = sc.tile([128, nun], dt.float32, tag="wmaxpart")
            for u in range(nun):
                wt = wstream.tile([128, 1024], dt.float32, tag="wstream")
                eng.dma_start(wt[:], ext[l, u])
                nc.vector.tensor_reduce(
                    part[:, u:u + 1], wt[:], axis=mybir.AxisListType.X,
                    op=ALU.max, apply_absolute_value=True,
                )
            nc.vector.tensor_reduce(
                wmax_cols[:, idx:idx + 1], part[:, 0:nun],
                axis=mybir.AxisListType.X, op=ALU.max,
                apply_absolute_value=False,
            )

        wsc0 = const.tile([128, 16], dt.float32, tag="wsc0")
        wsc1 = const.tile([128, 16], dt.float32, tag="wsc1")
        grow = const.tile([1, 16], dt.float32, tag="grow")
        mrow = const.tile([1, 16], dt.float32, tag="mrow")
        nc.gpsimd.memset(mrow[:], 0.0)
        wsc_row = const.tile([1, 16], dt.float32, tag="wscrow")

        def finish_wsc(cols, loc, glob, wsc):
            """partition-reduce wmax cols, allreduce across cores, build
            wsc[:, cols] = 127/(m+1e-5) broadcast to all partitions."""
            a, b = cols
            nc.gpsimd.tensor_reduce(
                mrow[:, a:b], wmax_cols[:, a:b], axis=mybir.AxisListType.C, op=ALU.max
            )
            nc.scalar.dma_start(loc[:], mrow[:])
            nc.gpsimd.collective_compute(
                "AllReduce",
                ALU.max,
                replica_groups=[list(range(n_cores))],
                ins=[loc[:].opt()],
                outs=[glob[:].opt()],
            )
            nc.scalar.dma_start(grow[:, a:b], glob[:, a:b])
            nc.vector.tensor_scalar(wsc_row[:, a:b], grow[:, a:b], 1e-5, None, op0=ALU.add)
            nc.vector.reciprocal(wsc_row[:, a:b], wsc_row[:, a:b])
            nc.vector.tensor_scalar(wsc_row[:, a:b], wsc_row[:, a:b], 127.0, None, op0=ALU.mult)
            pe_broadcast(wsc, wsc_row, 16)

        for mi, ext in enumerate((slg_ext, slu_ext, sld_ext)):
            slab_max(0, ext, mi, nc.scalar)
        finish_wsc((0, 3), mx0_loc, mx0_glob, wsc0)

        def emit_tail_wmax():
            # layers 1-3, emitted after layer 0's ternarize so the preamble
            # critical path only carries layer 0's slices + collective
            for l in range(1, n_layers):
                for mi, ext in enumerate((slg_ext, slu_ext, sld_ext)):
                    slab_max(l, ext, 3 * l + mi, nc.sync)
            finish_wsc((3, 3 * n_layers), mx1_loc, mx1_glob, wsc1)

        # ---------- ternarize ----------
        def tern_unit(src, dst_ap, idx, pass2_eng):
            """src: [128, 1024] fp32 AP; dst_ap: fp8 [128, 1024]-shaped."""
            wt = wstream.tile([128, 1024], dt.float32, tag="wstream")
            nc.sync.dma_start(wt[:], src)
            r32 = wi32.tile([128, 1024], dt.int32, tag="wi32")
            wsc = wsc0 if idx < 3 else wsc1
            nc.vector.tensor_scalar(
                r32[:], wt[:], wsc[:, idx:idx + 1], None, op0=ALU.mult
            )
            if pass2_eng == "act":
                # sign(n) == clip(n, -1, 1) for integer n
                nc.scalar.activation(dst_ap, r32[:], AF.Sign)
            else:
                nc.gpsimd.tensor_scalar(dst_ap, r32[:], 1, -1, op0=ALU.min, op1=ALU.max)

        def tern_layer(l, dst_g, dst_u, dst_d, via_dram, split_pass2=False):
            """dst_g/dst_u: fn(ch, dk) -> fp8 [128,1024] AP; dst_d: fn(ft) -> AP."""
            cnt = [0]

            def unit(src, dst_ap, idx):
                eng = "act" if (split_pass2 and cnt[0] % 2 == 0) else "gp"
                cnt[0] += 1
                if via_dram:
                    stg = w8s.tile([128, 1024], dt.float8e4, tag="w8s")
                    tern_unit(src, stg[:], idx, eng)
                    nc.sync.dma_start(dst_ap, stg[:])
                else:
                    tern_unit(src, dst_ap, idx, eng)

            for ch in range(NCH):
                for dk in range(NDK):
                    unit(wg_ext[l, ch, dk], dst_g(ch, dk), 3 * l)
                    unit(wu_ext[l, ch, dk], dst_u(ch, dk), 3 * l + 1)
            for ft in range(NFT):
                unit(wd_ext[l, ft * 128:(ft + 1) * 128, :], dst_d(ft), 3 * l + 2)

        # ---------- per-layer stats ----------
        # ssq/mx filled per token tile; the scalar chain runs on column
        # groups [0:4] and [4:8] as soon as those tiles' stats exist.
        def stats_chain(ssq_all, mx_all, c1_all, rs_all, a, b):
            k = b - a
            ms_t = btmp.tile([128, n_tok_tiles], dt.float32, tag="ms")
            rt_t = btmp.tile([128, n_tok_tiles], dt.float32, tag="rt")
            rstd_t = btmp.tile([128, n_tok_tiles], dt.float32, tag="rstd")
            nwt_t = btmp.tile([128, n_tok_tiles], dt.float32, tag="nwt")
            maxh_t = btmp.tile([128, n_tok_tiles], dt.float32, tag="maxh")
            sr_t = btmp.tile([128, n_tok_tiles], dt.float32, tag="sr")
            s_t = btmp.tile([128, n_tok_tiles], dt.float32, tag="s_")
            ms, rt, rstd = ms_t[:, 0:k], rt_t[:, 0:k], rstd_t[:, 0:k]
            nwt, maxh, sr, s_ = nwt_t[:, 0:k], maxh_t[:, 0:k], sr_t[:, 0:k], s_t[:, 0:k]
            nc.vector.tensor_scalar(ms, ssq_all[:, a:b], 1.0 / D, 1e-6, op0=ALU.mult, op1=ALU.add)
            nc.scalar.activation(rt, ms, AF.Sqrt)
            nc.vector.reciprocal(rstd, rt)
            # one Newton step: rstd *= 1.5 - 0.5*ms*rstd^2  (fixes the ~7e-6
            # Sqrt-LUT error that quantization tie-flips amplify layer by layer)
            nc.vector.tensor_tensor(nwt, rstd, rstd, op=ALU.mult)
            nc.vector.tensor_tensor(nwt, nwt, ms, op=ALU.mult)
            nc.vector.tensor_scalar(nwt, nwt, -0.5, 1.5, op0=ALU.mult, op1=ALU.add)
            nc.vector.tensor_tensor(rstd, rstd, nwt, op=ALU.mult)
            nc.vector.tensor_tensor(maxh, mx_all[:, a:b], rstd, op=ALU.mult)
            nc.vector.tensor_scalar(maxh, maxh, 1e-5, None, op0=ALU.add)
            nc.vector.reciprocal(sr, maxh)
            nc.vector.tensor_scalar(s_, sr, 127.0, 1e3, op0=ALU.mult, op1=ALU.min)
            nc.vector.tensor_scalar(s_, s_, 1e-3, None, op0=ALU.max)
            nc.vector.tensor_tensor(c1_all[:, a:b], s_, rstd, op=ALU.mult)
            nc.vector.reciprocal(rs_all[:, a:b], s_)

        def tile_stats(src_ap, ssq_all, mx_all, i, scale_bc):
            """ssq + scaled abs-max for token tile i of the NEXT layer's input."""
            scr = scrpool.tile([128, D], dt.float32, tag="scr")
            nc.scalar.activation(scr[:], src_ap, AF.Square, accum_out=ssq_all[:, i:i + 1])
            if is_ones:
                nc.vector.tensor_reduce(
                    mx_all[:, i:i + 1], src_ap, axis=mybir.AxisListType.X,
                    op=ALU.max, apply_absolute_value=True,
                )
            else:
                nc.vector.tensor_tensor(scr[:], src_ap, scale_bc[:], op=ALU.mult)
                nc.vector.tensor_reduce(
                    mx_all[:, i:i + 1], scr[:], axis=mybir.AxisListType.X,
                    op=ALU.max, apply_absolute_value=True,
                )

        # ---------- layers ----------
        xsrc = x_ext
        w8_next = None
        prev_stats = None  # (ssq, mx, c1, rs) for current layer, from fused pass

        for l in range(n_layers):
            wg_c, wu_c = [], []
            for _ch in range(NCH):
                wgc_t = wch.tile([128, NDK, 1024], dt.float8e4, tag="wch")
                wg_c.append(wgc_t)
            for _ch in range(NCH):
                wuc_t = wch.tile([128, NDK, 1024], dt.float8e4, tag="wch")
                wu_c.append(wuc_t)
            wd_t = wdp.tile([128, NFT, D], dt.float8e4, tag="wdp")
            if l == 0:
                tern_layer(
                    0,
                    lambda ch, dk: wg_c[ch][:, dk, :],
                    lambda ch, dk: wu_c[ch][:, dk, :],
                    lambda ft: wd_t[:, ft, :],
                    via_dram=False,
                    split_pass2=True,
                )
            else:
                wg8, wu8, wd8 = w8_next
                # fill in consumption order: wg/wu chunk-columns, then wd
                for ch in range(NCH):
                    nc.sync.dma_start(
                        wg_c[ch][:], wg8[:, :, ch * 1024:(ch + 1) * 1024])
                    nc.sync.dma_start(
                        wu_c[ch][:], wu8[:, :, ch * 1024:(ch + 1) * 1024])
                nc.sync.dma_start(wd_t[:], wd8[:])

            scale_bc = None
            scale_bc_next = None
            if not is_ones:
                rs_row = const.tile([1, D], dt.float32, tag=f"rsrow{l}")
                nc.sync.dma_start(rs_row[:], rs_ext[l:l + 1, :])
                scale_bc = bcpool.tile([128, D], dt.float32, tag="bc")
                pe_broadcast(scale_bc, rs_row[:], D)
                if l + 1 < n_layers:
                    rs_row_n = const.tile([1, D], dt.float32, tag=f"rsrown{l}")
                    nc.sync.dma_start(rs_row_n[:], rs_ext[l + 1:l + 2, :])
                    scale_bc_next = bcpool.tile([128, D], dt.float32, tag="bcn")
                    pe_broadcast(scale_bc_next, rs_row_n[:], D)

            if l == 0:
                # phase A for layer 0 only: standalone stats pass
                ssq_all = batch.tile([128, n_tok_tiles], dt.float32, tag="ssq")
                mx_all = batch.tile([128, n_tok_tiles], dt.float32, tag="mx")
                c1_all = batch.tile([128, n_tok_tiles], dt.float32, tag="c1")
                rs_all = batch.tile([128, n_tok_tiles], dt.float32, tag="rs_all")
                for i in range(n_tok_tiles):
                    xa = xpool.tile([128, D], dt.float32, tag="x1")
                    nc.scalar.dma_start(xa[:], xsrc[i * 128:(i + 1) * 128, :])
                    tile_stats(xa[:], ssq_all, mx_all, i, scale_bc)
                    if i == 3:
                        stats_chain(ssq_all, mx_all, c1_all, rs_all, 0, 4)
                    elif i == n_tok_tiles - 1:
                        stats_chain(ssq_all, mx_all, c1_all, rs_all, 4, n_tok_tiles)
            else:
                ssq_all, mx_all, c1_all, rs_all = prev_stats

            if l + 1 < n_layers:
                ssq_n = batch.tile([128, n_tok_tiles], dt.float32, tag="ssq")
                mx_n = batch.tile([128, n_tok_tiles], dt.float32, tag="mx")
                c1_n = batch.tile([128, n_tok_tiles], dt.float32, tag="c1")
                rs_n = batch.tile([128, n_tok_tiles], dt.float32, tag="rs_all")
                prev_stats = (ssq_n, mx_n, c1_n, rs_n)

            xdst = out_ext if l == n_layers - 1 else dram.tile([tok, D], dt.float32, tag="xbuf")

            # per-tile state
            stq = [None] * n_tok_tiles   # (x1, qT) from emit_q
            std = [None] * n_tok_tiles   # (x1, gqT, rs2) from emit_mm

            def emit_q(i):
                """load x tile, quantize, transpose -- one tile ahead of the MMs."""
                x1 = xpool.tile([128, D], dt.float32, tag="x1")
                nc.scalar.dma_start(x1[:], xsrc[i * 128:(i + 1) * 128, :])
                t1 = t1pool.tile([128, D], dt.float32, tag="t1")
                # q = round(c1 * h') via magic add (ACT) / subtract (DVE), out bf16
                if is_ones:
                    nc.scalar.activation(t1[:], x1[:], AF.Identity,
                                         scale=c1_all[:, i:i + 1], bias=mag[:])
                else:
                    nc.vector.tensor_tensor(t1[:], x1[:], scale_bc[:], op=ALU.mult)
                    nc.scalar.activation(t1[:], t1[:], AF.Identity,
                                         scale=c1_all[:, i:i + 1], bias=mag[:])
                q = qpool.tile([128, D], dt.bfloat16, tag="q")
                nc.vector.tensor_scalar(q[:], t1[:], MAGIC, None, op0=ALU.subtract)
                qT = qtpool.tile([128, NDK, 128], dt.bfloat16, tag="qt")
                nc.scalar.dma_start_transpose(qT[:], q[:])
                stq[i] = qT

            def emit_mm(i):
                """up-proj matmuls + GLU + act-quant + gqT for tile i."""
                qT = stq[i]
                g = gpool.tile([128, F], dt.float32, tag="g")
                gm8 = sc.tile([128, NFC], dt.float32, tag="gm8")
                for f in range(NFC):
                    ch = f // 2
                    fo = (f % 2) * 512
                    u_ps = ps_up.tile([128, 512], dt.float32, tag="ups")
                    v_ps = ps_up.tile([128, 512], dt.float32, tag="ups")
                    for dk in range(NDK):
                        nc.tensor.matmul(
                            u_ps[:], qT[:, dk, :], wg_c[ch][:, dk, fo:fo + 512],
                            start=(dk == 0), stop=(dk == NDK - 1),
                        )
                        nc.tensor.matmul(
                            v_ps[:], qT[:, dk, :], wu_c[ch][:, dk, fo:fo + 512],
                            start=(dk == 0), stop=(dk == NDK - 1),
                        )
                    su = silupool.tile([128, 512], dt.float32, tag="silu")
                    nc.scalar.activation(su[:], u_ps[:], AF.Silu,
                                         scale=rs_all[:, i:i + 1])
                    nc.vector.tensor_tensor(
                        g[:, f * 512:(f + 1) * 512], su[:], v_ps[:], op=ALU.mult
                    )
                    nc.vector.tensor_reduce(
                        gm8[:, f:f + 1], g[:, f * 512:(f + 1) * 512],
                        axis=mybir.AxisListType.X, op=ALU.max, apply_absolute_value=True,
                    )
                # s2 = clip(127/(max|g|/s + 1e-5)); c2 = s2/s ; rs2 = 1/s2
                gmx = sc.tile([128, 1], dt.float32, tag="gmx")
                nc.vector.tensor_reduce(
                    gmx[:], gm8[:], axis=mybir.AxisListType.X, op=ALU.max,
                    apply_absolute_value=False,
                )
                nc.vector.tensor_tensor(gmx[:], gmx[:], rs_all[:, i:i + 1], op=ALU.mult)
                nc.vector.tensor_scalar(gmx[:], gmx[:], 1e-5, None, op0=ALU.add)
                s2r = sc.tile([128, 1], dt.float32, tag="s2r")
                nc.vector.reciprocal(s2r[:], gmx[:])
                s2 = sc.tile([128, 1], dt.float32, tag="s2")
                nc.vector.tensor_scalar(s2[:], s2r[:], 127.0, 1e3, op0=ALU.mult, op1=ALU.min)
                nc.vector.tensor_scalar(s2[:], s2[:], 1e-3, None, op0=ALU.max)
                c2 = sc.tile([128, 1], dt.float32, tag="c2")
                nc.vector.tensor_tensor(c2[:], s2[:], rs_all[:, i:i + 1], op=ALU.mult)
                rs2 = sc.tile([128, 1], dt.float32, tag="rs2")
                nc.vector.reciprocal(rs2[:], s2[:])
                # gq = round(c2*g) in two halves (ACT magic pass in place, DVE
                # subtract to bf16, transpose) so the serial tail is half as long
                gq = gqpool.tile([128, F], dt.bfloat16, tag="gq")
                gqT = gqtpool.tile([128, NFT, 128], dt.bfloat16, tag="gqt")
                for h in range(2):
                    hs = h * (F // 2)
                    he = hs + F // 2
                    nc.scalar.activation(g[:, hs:he], g[:, hs:he], AF.Identity,
                                         scale=c2[:], bias=mag[:])
                    nc.vector.tensor_scalar(gq[:, hs:he], g[:, hs:he], MAGIC, None,
                                            op0=ALU.subtract)
                    nc.scalar.dma_start_transpose(
                        gqT[:, h * (NFT // 2):(h + 1) * (NFT // 2), :], gq[:, hs:he])
                std[i] = (gqT, rs2)
                stq[i] = None

            def emit_down(i):
                gqT, rs2 = std[i]
                xr = xrpool.tile([128, D], dt.float32, tag="xr")
                nc.scalar.dma_start(xr[:], xsrc[i * 128:(i + 1) * 128, :])
                xd_ps0 = ps_dn.tile([128, 512], dt.float32, tag="dps")
                xd_ps1 = ps_dn.tile([128, 512], dt.float32, tag="dps")
                for ft in range(NFT):
                    nc.tensor.matmul(
                        xd_ps0[:], gqT[:, ft, :], wd_t[:, ft, 0:512],
                        start=(ft == 0), stop=(ft == NFT - 1),
                    )
                    nc.tensor.matmul(
                        xd_ps1[:], gqT[:, ft, :], wd_t[:, ft, 512:1024],
                        start=(ft == 0), stop=(ft == NFT - 1),
                    )
                xnew = s4k.tile([128, D], dt.float32, tag="s4k")
                for dc, xd_ps in ((0, xd_ps0), (1, xd_ps1)):
                    xdr = xdrpool.tile([128, 512], dt.float32, tag="xdr")
                    nc.scalar.activation(xdr[:], xd_ps[:], AF.Copy, scale=rs2[:])
                    nc.vector.tensor_tensor(
                        xnew[:, dc * 512:(dc + 1) * 512],
                        xr[:, dc * 512:(dc + 1) * 512], xdr[:], op=ALU.add,
                    )
                if l + 1 < n_layers:
                    tile_stats(xnew[:], ssq_n, mx_n, i, scale_bc_next)
                    if i == 3:
                        stats_chain(ssq_n, mx_n, c1_n, rs_n, 0, 4)
                    elif i == n_tok_tiles - 1:
                        stats_chain(ssq_n, mx_n, c1_n, rs_n, 4, n_tok_tiles)
                nc.scalar.dma_start(xdst[i * 128:(i + 1) * 128, :], xnew[:])
                std[i] = None

            # software pipeline with one-tile lookahead on q/qT:
            # q0 q1 | mm0 q2 | mm1 q3 dn0 | mm2 q4 dn1 | ... | mm7 dn6 | dn7
            emit_q(0)
            emit_q(1)
            for i in range(n_tok_tiles):
                emit_mm(i)
                if i + 2 < n_tok_tiles:
                    emit_q(i + 2)
                if i >= 1:
                    emit_down(i - 1)
                if l == 0 and i == 1:
                    emit_tail_wmax()
            emit_down(n_tok_tiles - 1)

            xsrc = xdst
            if l + 1 < n_layers:
                wg8 = dram.tile([128, NDK, F], dt.float8e4, tag="wg8")
                wu8 = dram.tile([128, NDK, F], dt.float8e4, tag="wu8")
                wd8 = dram.tile([128, NFT, D], dt.float8e4, tag="wd8")
                w8_next = (wg8, wu8, wd8)
                tern_layer(
                    l + 1,
                    lambda ch, dk: wg8[:, dk, ch * 1024:(ch + 1) * 1024],
                    lambda ch, dk: wu8[:, dk, ch * 1024:(ch + 1) * 1024],
                    lambda ft: wd8[:, ft, :],
                    via_dram=True,
                )

    _split_excess_waits(nc)
    return nc


_nc_cache = {}


def _get_nc(key):
    if key not in _nc_cache:
        _nc_cache[key] = build(*key)
    return _nc_cache[key]


def _repack(w, n_layers):
    # [L, D, F] -> [L, F//1024, D//128, 128, 1024] so each (ch, dk) tile is
    # contiguous; dk == core index for the per-core max slices.
    return np.ascontiguousarray(
        w.reshape(n_layers, D // 128, 128, NCH, 1024).transpose(0, 3, 1, 2, 4)
    )


def _make_in_maps(x, rs, wg, wu, wd, n_cores=NCORES):
    n_layers = rs.shape[0]
    wg_r = _repack(wg, n_layers)
    wu_r = _repack(wu, n_layers)
    nsl = NDK // n_cores  # dk-slabs per core for slg/slu
    dsl = F // n_cores
    in_maps = []
    for c in range(n_cores):
        slg = wg_r[:, :, c * nsl:(c + 1) * nsl].reshape(n_layers, -1, 128, 1024)
        slu = wu_r[:, :, c * nsl:(c + 1) * nsl].reshape(n_layers, -1, 128, 1024)
        sld = wd[:, c * dsl:(c + 1) * dsl, :].reshape(n_layers, -1, 128, 1024)
        in_maps.append({
            "x": x[c],
            "rs": rs,
            "wg": wg_r,
            "wu": wu_r,
            "wd": wd,
            "slg": np.ascontiguousarray(slg),
            "slu": np.ascontiguousarray(slu),
            "sld": np.ascontiguousarray(sld),
        })
    return in_maps


def kernel(x, rms_scale, W_g, W_u, W_d):
    """Full-input entry point: shard over batch, run 8-core SPMD, gather."""
    x = np.ascontiguousarray(np.asarray(x, dtype=np.float32))
    rs = np.ascontiguousarray(np.asarray(rms_scale, dtype=np.float32))
    wg = np.ascontiguousarray(np.asarray(W_g, dtype=np.float32))
    wu = np.ascontiguousarray(np.asarray(W_u, dtype=np.float32))
    wd = np.ascontiguousarray(np.asarray(W_d, dtype=np.float32))
    B, S, Dx = x.shape
    assert (B, S, Dx) == (NCORES, TOK, D), (B, S, Dx)
    is_ones = bool(np.all(rs == 1.0))
    nc = _get_nc((is_ones,))
    in_maps = _make_in_maps(x, rs, wg, wu, wd)
    res = run_bass_kernel_spmd(nc, in_maps, list(range(NCORES)))
    return np.stack([res.results[c]["out"] for c in range(NCORES)], axis=0)


# revision 14
# speedup vs baseline: 1.0088x; 1.0088x over previous
"""Trainium2 Bass kernel for nn_CascadeTransformerMM (4-layer ternary-GLU cascade).

Math (per layer, per token row):
  h   = rms_scale * x * rsqrt(mean(x^2) + 1e-6)
  s   = clip(127/(max|h| + 1e-5), 1e-3, 1e3);  q = round(s*h)      (ints in [-127,127])
  Wt  = clip(round(W * 127/(max|W| + 1e-5)), -1, 1)                 (ternary {-1,0,1})
  u   = (q @ Wg_t)/s ; v = (q @ Wu_t)/s ; g = silu(u)*v
  s2  = clip(127/(max|g| + 1e-5), 1e-3, 1e3); gq = round(s2*g)
  x  += (gq @ Wd_t)/s2

Distribution: pure data-parallel over the batch dim (8 batches -> 8 cores),
weights replicated per core. Per-matrix |W|max is computed cooperatively:
each core reduces a 1/8 row-slice, then a tiny AllReduce(max) shares the 12
scalars (layer 0's three matrices allreduced first so its ternarize can
start early). All matmuls run on the PE array with bf16 activations
(integers <= 127, exact) x fp8 ternary weights (exact), fp32 PSUM
accumulation -> the heavy compute is bit-exact integer arithmetic.

Schedule: per-token-tile software pipeline with one-tile lookahead --
q/qT production for tile i+2 and down-proj for tile i-1 are emitted
around the up-proj matmul block of tile i, so the serial per-tile tail
(s2 chain -> gq round -> gqT transpose) hides under the next tile's
matmuls and the PE stream stays dense (keeps the HAM clock-gate warm).
Weights live in per-column-chunk SBUF tiles so layer-0 ternarize and
layer-boundary refills unblock the matmul stream progressively.
Ternarize runs pass 1 (scale+round-to-int32) on DVE and pass 2 (clip to
{-1,0,1} as fp8) on the otherwise-idle GPSIMD (alternating with ACT
Sign for layer 0, where latency matters)."""

import os
import sys

for _p in ("/opt/trn_rl_repo", "/root/.axon_site/_ro/trn_rl_repo"):
    if os.path.isdir(_p) and _p not in sys.path:
        sys.path.insert(0, _p)

import numpy as np
from contextlib import ExitStack

import concourse.bass as bass
import concourse.bass_isa as bass_isa
import concourse.mybir as mybir
import concourse.tile as tile
from concourse.bass_utils import run_bass_kernel_spmd

dt = mybir.dt
AF = mybir.ActivationFunctionType
ALU = mybir.AluOpType

MAGIC = float(1.5 * 2**23)  # fp32 round-to-nearest-even magic constant
D = 1024
F = 4096
L = 4
NCORES = 8
TOK = 1024  # tokens per core (one batch of S=1024)

NDK = D // 128   # 8 contraction tiles for up-proj
NFT = F // 128   # 32 contraction tiles for down-proj
NFC = F // 512   # 8 free-dim chunks for up-proj
NCH = F // 1024  # 4 column-chunks in the repacked wg/wu layout
NTT = TOK // 128  # 8 token tiles


def _split_excess_waits(nc, max_waits: int = 1) -> int:
    """walrus in this container rejects >1 sync-wait per instruction; split
    extras into standalone event-semaphore waits on the same engine (same-
    engine program order makes this semantically identical)."""
    n = 0
    for func in nc.m.functions:
        for block in func.blocks:
            changed = False
            out = []
            for inst in block.instructions:
                si = getattr(inst, "sync_info", None)
                if si is not None and si.on_wait and len(si.on_wait) > max_waits:
                    waits = list(si.on_wait)
                    for j, w in enumerate(waits[max_waits:]):
                        out.append(
                            mybir.InstEventSemaphore(
                                name=f"{inst.name}-xw{j}",
                                engine=inst.engine,
                                ins=[],
                                outs=[],
                                sync_info=mybir.SyncInfo(on_wait=[w], on_update=[]),
                            )
                        )
                        n += 1
                    inst.sync_info = mybir.SyncInfo(
                        on_wait=waits[:max_waits], on_update=list(si.on_update)
                    )
                    changed = True
                out.append(inst)
            if changed:
                block.instructions = out
    return n


def build(is_ones: bool = True, n_cores: int = NCORES, n_tok_tiles: int = NTT,
          n_layers: int = L) -> bass.Bass:
    nc = bass.Bass(num_devices=n_cores)
    tok = n_tok_tiles * 128

    x_ext = nc.declare_dram_parameter("x", [tok, D], dt.float32, isOutput=False)
    rs_ext = nc.declare_dram_parameter("rs", [n_layers, D], dt.float32, isOutput=False)
    # wg/wu repacked host-side to [L, F//1024, NDK, 128, 1024] so every
    # [128, 1024] weight tile is one contiguous 512 KB DMA; wd's row-slabs
    # are naturally contiguous.
    wg_ext = nc.declare_dram_parameter("wg", [n_layers, NCH, NDK, 128, 1024], dt.float32, isOutput=False)
    wu_ext = nc.declare_dram_parameter("wu", [n_layers, NCH, NDK, 128, 1024], dt.float32, isOutput=False)
    wd_ext = nc.declare_dram_parameter("wd", [n_layers, F, D], dt.float32, isOutput=False)
    # per-core row-slices of each matrix for the cooperative |W|max,
    # as [L, nun, 128, 1024] contiguous units
    nun = (D // n_cores) * F // (128 * 1024)
    slg_ext = nc.declare_dram_parameter("slg", [n_layers, nun, 128, 1024], dt.float32, isOutput=False)
    slu_ext = nc.declare_dram_parameter("slu", [n_layers, nun, 128, 1024], dt.float32, isOutput=False)
    sld_ext = nc.declare_dram_parameter("sld", [n_layers, nun, 128, 1024], dt.float32, isOutput=False)
    out_ext = nc.declare_dram_parameter("out", [tok, D], dt.float32, isOutput=True)

    mx0_loc = nc.dram_tensor("mx0_loc", [1, 16], dt.float32)
    mx0_glob = nc.dram_tensor("mx0_glob", [1, 16], dt.float32)
    mx1_loc = nc.dram_tensor("mx1_loc", [1, 16], dt.float32)
    mx1_glob = nc.dram_tensor("mx1_glob", [1, 16], dt.float32)

    with tile.TileContext(nc) as tc, ExitStack() as ctx:
        P = ctx.enter_context
        wch = P(tc.tile_pool(name="wch", bufs=2 * NCH))   # per-ch wg/wu fp8 tiles
        wdp = P(tc.tile_pool(name="wdp", bufs=1))         # wd fp8 tile
        wstream = P(tc.tile_pool(name="wstream", bufs=2))
        wi32 = P(tc.tile_pool(name="wi32", bufs=2))
        xpool = P(tc.tile_pool(name="x1", bufs=5))
        t1pool = P(tc.tile_pool(name="t1", bufs=1))
        s4k = P(tc.tile_pool(name="s4k", bufs=2))         # xnew scratch
        qpool = P(tc.tile_pool(name="q", bufs=2))
        qtpool = P(tc.tile_pool(name="qt", bufs=2))
        silupool = P(tc.tile_pool(name="silu", bufs=2))   # [128,512] silu chunks
        scrpool = P(tc.tile_pool(name="scr", bufs=1))     # square scratch
        gpool = P(tc.tile_pool(name="g", bufs=1))
        gqpool = P(tc.tile_pool(name="gq", bufs=1))
        gqtpool = P(tc.tile_pool(name="gqt", bufs=2))
        w8s = P(tc.tile_pool(name="w8s", bufs=1))
        batch = P(tc.tile_pool(name="batch", bufs=2))     # [128, ntt] per-layer stats
        btmp = P(tc.tile_pool(name="btmp", bufs=1))       # stats chain temps
        sc = P(tc.tile_pool(name="sc", bufs=4))           # [128, small] scalars
        xdrpool = P(tc.tile_pool(name="xdr", bufs=1))
        const = P(tc.tile_pool(name="const", bufs=1))
        bcpool = P(tc.tile_pool(name="bc", bufs=2))       # rms_scale broadcast (general path)
        dram = P(tc.tile_pool(name="dram", bufs=2, space="DRAM"))
        ps_up = P(tc.tile_pool(name="psup", bufs=4, space="PSUM"))
        ps_dn = P(tc.tile_pool(name="psdn", bufs=4, space="PSUM"))

        # ---------- constants ----------
        mag = const.tile([128, 1], dt.float32, tag="mag")
        nc.gpsimd.memset(mag[:], MAGIC)
        ones1 = const.tile([1, 128], dt.float32, tag="ones1")
        nc.gpsimd.memset(ones1[:], 1.0)

        def pe_broadcast(dst, src_row, n):
            """broadcast src_row [1, n] to dst [128, n] via PE outer product."""
            for h in range(0, n, 512):
                w = min(512, n - h)
                bc_ps = ps_up.tile([128, 512], dt.float32, tag="ups")
                nc.tensor.matmul(bc_ps[:, 0:w], ones1[:], src_row[:, h:h + w],
                                 start=True, stop=True)
                nc.scalar.activation(dst[:, h:h + w], bc_ps[:, 0:w], AF.Copy)

        # ---------- cooperative per-matrix |W|max ----------
        # Layer 0's three matrices are reduced + allreduced first so its
        # ternarize can start while layers 1-3 slices still stream.
        wmax_cols = const.tile([128, 16], dt.float32, tag="wmaxc")
        nc.gpsimd.memset(wmax_cols[:], 0.0)

        def slab_max(l, ext, idx, eng):
            part = sc.tile([128, nun], dt.float32, tag="wmaxpart")
            for u in range(nun):
                wt = wstream.tile([128, 1024], dt.float32, tag="wstream")
                eng.dma_start(wt[:], ext[l, u])
                nc.vector.tensor_reduce(
                    part[:, u:u + 1], wt[:], axis=mybir.AxisListType.X,
                    op=ALU.max, apply_absolute_value=True,
                )
            nc.vector.tensor_reduce(
                wmax_cols[:, idx:idx + 1], part[:, 0:nun],
                axis=mybir.AxisListType.X, op=ALU.max,
                apply_absolute_value=False,
            )

        wsc0 = const.tile([128, 16], dt.float32, tag="wsc0")
        wsc1 = const.tile([128, 16], dt.float32, tag="wsc1")
        grow = const.tile([1, 16], dt.float32, tag="grow")
        mrow = const.tile([1, 16], dt.float32, tag="mrow")
        nc.gpsimd.memset(mrow[:], 0.0)
        wsc_row = const.tile([1, 16], dt.float32, tag="wscrow")

        def finish_wsc(cols, loc, glob, wsc):
            """partition-reduce wmax cols, allreduce across cores, build
            wsc[:, cols] = 127/(m+1e-5) broadcast to all partitions."""
            a, b = cols
            nc.gpsimd.tensor_reduce(
                mrow[:, a:b], wmax_cols[:, a:b], axis=mybir.AxisListType.C, op=ALU.max
            )
            nc.scalar.dma_start(loc[:], mrow[:])
            nc.gpsimd.collective_compute(
                "AllReduce",
                ALU.max,
                replica_groups=[list(range(n_cores))],
                ins=[loc[:].opt()],
                outs=[glob[:].opt()],
            )
            nc.scalar.dma_start(grow[:, a:b], glob[:, a:b])
            nc.vector.tensor_scalar(wsc_row[:, a:b], grow[:, a:b], 1e-5, None, op0=ALU.add)
            nc.vector.reciprocal(wsc_row[:, a:b], wsc_row[:, a:b])
            nc.vector.tensor_scalar(wsc_row[:, a:b], wsc_row[:, a:b], 127.0, None, op0=ALU.mult)
            pe_broadcast(wsc, wsc_row, 16)

        for mi, ext in enumerate((slg_ext, slu_ext, sld_ext)):
            slab_max(0, ext, mi, nc.scalar)
        finish_wsc((0, 3), mx0_loc, mx0_glob, wsc0)

        def emit_tail_wmax():
            # layers 1-3, emitted after layer 0's ternarize so the preamble
            # critical path only carries layer 0's slices + collective
            for l in range(1, n_layers):
                for mi, ext in enumerate((slg_ext, slu_ext, sld_ext)):
                    slab_max(l, ext, 3 * l + mi, nc.sync)
            finish_wsc((3, 3 * n_layers), mx1_loc, mx1_glob, wsc1)

        # ---------- ternarize ----------
        def tern_unit(src, dst_ap, idx, pass2_eng):
            """src: [128, 1024] fp32 AP; dst_ap: fp8 [128, 1024]-shaped."""
            wt = wstream.tile([128, 1024], dt.float32, tag="wstream")
            nc.sync.dma_start(wt[:], src)
            r32 = wi32.tile([128, 1024], dt.int32, tag="wi32")
            wsc = wsc0 if idx < 3 else wsc1
            nc.vector.tensor_scalar(
                r32[:], wt[:], wsc[:, idx:idx + 1], None, op0=ALU.mult
            )
            if pass2_eng == "act":
                # sign(n) == clip(n, -1, 1) for integer n
                nc.scalar.activation(dst_ap, r32[:], AF.Sign)
            else:
                nc.gpsimd.tensor_scalar(dst_ap, r32[:], 1, -1, op0=ALU.min, op1=ALU.max)

        def tern_layer(l, dst_g, dst_u, dst_d, via_dram, split_pass2=False):
            """dst_g/dst_u: fn(ch, dk) -> fp8 [128,1024] AP; dst_d: fn(ft) -> AP."""
            cnt = [0]

            def unit(src, dst_ap, idx):
                eng = "act" if (split_pass2 and cnt[0] % 2 == 0) else "gp"
                cnt[0] += 1
                if via_dram:
                    stg = w8s.tile([128, 1024], dt.float8e4, tag="w8s")
                    tern_unit(src, stg[:], idx, eng)
                    nc.sync.dma_start(dst_ap, stg[:])
                else:
                    tern_unit(src, dst_ap, idx, eng)

            for ch in range(NCH):
                for dk in range(NDK):
                    unit(wg_ext[l, ch, dk], dst_g(ch, dk), 3 * l)
                    unit(wu_ext[l, ch, dk], dst_u(ch, dk), 3 * l + 1)
            for ft in range(NFT):
                unit(wd_ext[l, ft * 128:(ft + 1) * 128, :], dst_d(ft), 3 * l + 2)

        # ---------- per-layer stats ----------
        # ssq/mx filled per token tile; the scalar chain runs on column
        # groups [0:4] and [4:8] as soon as those tiles' stats exist.
        def stats_chain(ssq_all, mx_all, c1_all, rs_all, a, b):
            k = b - a
            ms_t = btmp.tile([128, n_tok_tiles], dt.float32, tag="ms")
            rt_t = btmp.tile([128, n_tok_tiles], dt.float32, tag="rt")
            rstd_t = btmp.tile([128, n_tok_tiles], dt.float32, tag="rstd")
            nwt_t = btmp.tile([128, n_tok_tiles], dt.float32, tag="nwt")
            maxh_t = btmp.tile([128, n_tok_tiles], dt.float32, tag="maxh")
            sr_t = btmp.tile([128, n_tok_tiles], dt.float32, tag="sr")
            s_t = btmp.tile([128, n_tok_tiles], dt.float32, tag="s_")
            ms, rt, rstd = ms_t[:, 0:k], rt_t[:, 0:k], rstd_t[:, 0:k]
            nwt, maxh, sr, s_ = nwt_t[:, 0:k], maxh_t[:, 0:k], sr_t[:, 0:k], s_t[:, 0:k]
            nc.vector.tensor_scalar(ms, ssq_all[:, a:b], 1.0 / D, 1e-6, op0=ALU.mult, op1=ALU.add)
            nc.scalar.activation(rt, ms, AF.Sqrt)
            nc.vector.reciprocal(rstd, rt)
            # one Newton step: rstd *= 1.5 - 0.5*ms*rstd^2  (fixes the ~7e-6
            # Sqrt-LUT error that quantization tie-flips amplify layer by layer)
            nc.vector.tensor_tensor(nwt, rstd, rstd, op=ALU.mult)
            nc.vector.tensor_tensor(nwt, nwt, ms, op=ALU.mult)
            nc.vector.tensor_scalar(nwt, nwt, -0.5, 1.5, op0=ALU.mult, op1=ALU.add)
            nc.vector.tensor_tensor(rstd, rstd, nwt, op=ALU.mult)
            nc.vector.tensor_tensor(maxh, mx_all[:, a:b], rstd, op=ALU.mult)
            nc.vector.tensor_scalar(maxh, maxh, 1e-5, None, op0=ALU.add)
            nc.vector.reciprocal(sr, maxh)
            nc.vector.tensor_scalar(s_, sr, 127.0, 1e3, op0=ALU.mult, op1=ALU.min)
            nc.vector.tensor_scalar(s_, s_, 1e-3, None, op0=ALU.max)
            nc.vector.tensor_tensor(c1_all[:, a:b], s_, rstd, op=ALU.mult)
            nc.vector.reciprocal(rs_all[:, a:b], s_)

        def tile_stats(src_ap, ssq_all, mx_all, i, scale_bc):
            """ssq + scaled abs-max for token tile i of the NEXT layer's input."""
            scr = scrpool.tile([128, D], dt.float32, tag="scr")
            nc.scalar.activation(scr[:], src_ap, AF.Square, accum_out=ssq_all[:, i:i + 1])
            if is_ones:
                nc.vector.tensor_reduce(
                    mx_all[:, i:i + 1], src_ap, axis=mybir.AxisListType.X,
                    op=ALU.max, apply_absolute_value=True,
                )
            else:
                nc.vector.tensor_tensor(scr[:], src_ap, scale_bc[:], op=ALU.mult)
                nc.vector.tensor_reduce(
                    mx_all[:, i:i + 1], scr[:], axis=mybir.AxisListType.X,
                    op=ALU.max, apply_absolute_value=True,
                )

        # ---------- layers ----------
        xsrc = x_ext
        w8_next = None
        prev_stats = None  # (ssq, mx, c1, rs) for current layer, from fused pass

        for l in range(n_layers):
            wg_c, wu_c = [], []
            for _ch in range(NCH):
                wgc_t = wch.tile([128, NDK, 1024], dt.float8e4, tag="wch")
                wg_c.append(wgc_t)
            for _ch in range(NCH):
                wuc_t = wch.tile([128, NDK, 1024], dt.float8e4, tag="wch")
                wu_c.append(wuc_t)
            wd_t = wdp.tile([128, NFT, D], dt.float8e4, tag="wdp")
            if l == 0:
                tern_layer(
                    0,
                    lambda ch, dk: wg_c[ch][:, dk, :],
                    lambda ch, dk: wu_c[ch][:, dk, :],
                    lambda ft: wd_t[:, ft, :],
                    via_dram=False,
                    split_pass2=True,
                )
            else:
                wg8, wu8, wd8 = w8_next
                # fill in consumption order: wg/wu chunk-columns, then wd
                for ch in range(NCH):
                    nc.sync.dma_start(
                        wg_c[ch][:], wg8[:, :, ch * 1024:(ch + 1) * 1024])
                    nc.sync.dma_start(
                        wu_c[ch][:], wu8[:, :, ch * 1024:(ch + 1) * 1024])
                nc.sync.dma_start(wd_t[:], wd8[:])

            scale_bc = None
            scale_bc_next = None
            if not is_ones:
                rs_row = const.tile([1, D], dt.float32, tag=f"rsrow{l}")
                nc.sync.dma_start(rs_row[:], rs_ext[l:l + 1, :])
                scale_bc = bcpool.tile([128, D], dt.float32, tag="bc")
                pe_broadcast(scale_bc, rs_row[:], D)
                if l + 1 < n_layers:
                    rs_row_n = const.tile([1, D], dt.float32, tag=f"rsrown{l}")
                    nc.sync.dma_start(rs_row_n[:], rs_ext[l + 1:l + 2, :])
                    scale_bc_next = bcpool.tile([128, D], dt.float32, tag="bcn")
                    pe_broadcast(scale_bc_next, rs_row_n[:], D)

            if l == 0:
                # phase A for layer 0 only: standalone stats pass
                ssq_all = batch.tile([128, n_tok_tiles], dt.float32, tag="ssq")
                mx_all = batch.tile([128, n_tok_tiles], dt.float32, tag="mx")
                c1_all = batch.tile([128, n_tok_tiles], dt.float32, tag="c1")
                rs_all = batch.tile([128, n_tok_tiles], dt.float32, tag="rs_all")
                for i in range(n_tok_tiles):
                    xa = xpool.tile([128, D], dt.float32, tag="x1")
                    nc.scalar.dma_start(xa[:], xsrc[i * 128:(i + 1) * 128, :])
                    tile_stats(xa[:], ssq_all, mx_all, i, scale_bc)
                    if i == 3:
                        stats_chain(ssq_all, mx_all, c1_all, rs_all, 0, 4)
                    elif i == n_tok_tiles - 1:
                        stats_chain(ssq_all, mx_all, c1_all, rs_all, 4, n_tok_tiles)
            else:
                ssq_all, mx_all, c1_all, rs_all = prev_stats

            if l + 1 < n_layers:
                ssq_n = batch.tile([128, n_tok_tiles], dt.float32, tag="ssq")
                mx_n = batch.tile([128, n_tok_tiles], dt.float32, tag="mx")
                c1_n = batch.tile([128, n_tok_tiles], dt.float32, tag="c1")
                rs_n = batch.tile([128, n_tok_tiles], dt.float32, tag="rs_all")
                prev_stats = (ssq_n, mx_n, c1_n, rs_n)

            xdst = out_ext if l == n_layers - 1 else dram.tile([tok, D], dt.float32, tag="xbuf")

            # per-tile state
            stq = [None] * n_tok_tiles   # (x1, qT) from emit_q
            std = [None] * n_tok_tiles   # (x1, gqT, rs2) from emit_mm

            def emit_q(i):
                """load x tile, quantize, transpose -- one tile ahead of the MMs."""
                x1 = xpool.tile([128, D], dt.float32, tag="x1")
                nc.scalar.dma_start(x1[:], xsrc[i * 128:(i + 1) * 128, :])
                t1 = t1pool.tile([128, D], dt.float32, tag="t1")
                # q = round(c1 * h') via magic add (ACT) / subtract (DVE), out bf16
                if is_ones:
                    nc.scalar.activation(t1[:], x1[:], AF.Identity,
                                         scale=c1_all[:, i:i + 1], bias=mag[:])
                else:
                    nc.vector.tensor_tensor(t1[:], x1[:], scale_bc[:], op=ALU.mult)
                    nc.scalar.activation(t1[:], t1[:], AF.Identity,
                                         scale=c1_all[:, i:i + 1], bias=mag[:])
                q = qpool.tile([128, D], dt.bfloat16, tag="q")
                nc.vector.tensor_scalar(q[:], t1[:], MAGIC, None, op0=ALU.subtract)
                qT = qtpool.tile([128, NDK, 128], dt.bfloat16, tag="qt")
                nc.scalar.dma_start_transpose(qT[:], q[:])
                stq[i] = (x1, qT)

            def emit_mm(i):
                """up-proj matmuls + GLU + act-quant + gqT for tile i."""
                x1, qT = stq[i]
                g = gpool.tile([128, F], dt.float32, tag="g")
                gm8 = sc.tile([128, NFC], dt.float32, tag="gm8")
                for f in range(NFC):
                    ch = f // 2
                    fo = (f % 2) * 512
                    u_ps = ps_up.tile([128, 512], dt.float32, tag="ups")
                    v_ps = ps_up.tile([128, 512], dt.float32, tag="ups")
                    for dk in range(NDK):
                        nc.tensor.matmul(
                            u_ps[:], qT[:, dk, :], wg_c[ch][:, dk, fo:fo + 512],
                            start=(dk == 0), stop=(dk == NDK - 1),
                        )
                        nc.tensor.matmul(
                            v_ps[:], qT[:, dk, :], wu_c[ch][:, dk, fo:fo + 512],
                            start=(dk == 0), stop=(dk == NDK - 1),
                        )
                    su = silupool.tile([128, 512], dt.float32, tag="silu")
                    nc.scalar.activation(su[:], u_ps[:], AF.Silu,
                                         scale=rs_all[:, i:i + 1])
                    nc.vector.tensor_tensor(
                        g[:, f * 512:(f + 1) * 512], su[:], v_ps[:], op=ALU.mult
                    )
                    nc.vector.tensor_reduce(
                        gm8[:, f:f + 1], g[:, f * 512:(f + 1) * 512],
                        axis=mybir.AxisListType.X, op=ALU.max, apply_absolute_value=True,
                    )
                # s2 = clip(127/(max|g|/s + 1e-5)); c2 = s2/s ; rs2 = 1/s2
                gmx = sc.tile([128, 1], dt.float32, tag="gmx")
                nc.vector.tensor_reduce(
                    gmx[:], gm8[:], axis=mybir.AxisListType.X, op=ALU.max,
                    apply_absolute_value=False,
                )
                nc.vector.tensor_tensor(gmx[:], gmx[:], rs_all[:, i:i + 1], op=ALU.mult)
                nc.vector.tensor_scalar(gmx[:], gmx[:], 1e-5, None, op0=ALU.add)
                s2r = sc.tile([128, 1], dt.float32, tag="s2r")
                nc.vector.reciprocal(s2r[:], gmx[:])
                s2 = sc.tile([128, 1], dt.float32, tag="s2")
                nc.vector.tensor_scalar(s2[:], s2r[:], 127.0, 1e3, op0=ALU.mult, op1=ALU.min)
                nc.vector.tensor_scalar(s2[:], s2[:], 1e-3, None, op0=ALU.max)
                c2 = sc.tile([128, 1], dt.float32, tag="c2")
                nc.vector.tensor_tensor(c2[:], s2[:], rs_all[:, i:i + 1], op=ALU.mult)
                rs2 = sc.tile([128, 1], dt.float32, tag="rs2")
                nc.vector.reciprocal(rs2[:], s2[:])
                # gq = round(c2*g) in two halves (ACT magic pass in place, DVE
                # subtract to bf16, transpose) so the serial tail is half as long
                gq = gqpool.tile([128, F], dt.bfloat16, tag="gq")
                gqT = gqtpool.tile([128, NFT, 128], dt.bfloat16, tag="gqt")
                for h in range(2):
                    hs = h * (F // 2)
                    he = hs + F // 2
                    nc.scalar.activation(g[:, hs:he], g[:, hs:he], AF.Identity,
                                         scale=c2[:], bias=mag[:])
                    nc.vector.tensor_scalar(gq[:, hs:he], g[:, hs:he], MAGIC, None,
                                            op0=ALU.subtract)
                    nc.scalar.dma_start_transpose(
                        gqT[:, h * (NFT // 2):(h + 1) * (NFT // 2), :], gq[:, hs:he])
                std[i] = (x1, gqT, rs2)
                stq[i] = None

            def emit_down(i):
                x1, gqT, rs2 = std[i]
                xd_ps0 = ps_dn.tile([128, 512], dt.float32, tag="dps")
                xd_ps1 = ps_dn.tile([128, 512], dt.float32, tag="dps")
                for ft in range(NFT):
                    nc.tensor.matmul(
                        xd_ps0[:], gqT[:, ft, :], wd_t[:, ft, 0:512],
                        start=(ft == 0), stop=(ft == NFT - 1),
                    )
                    nc.tensor.matmul(
                        xd_ps1[:], gqT[:, ft, :], wd_t[:, ft, 512:1024],
                        start=(ft == 0), stop=(ft == NFT - 1),
                    )
                xnew = s4k.tile([128, D], dt.float32, tag="s4k")
                for dc, xd_ps in ((0, xd_ps0), (1, xd_ps1)):
                    xdr = xdrpool.tile([128, 512], dt.float32, tag="xdr")
                    nc.scalar.activation(xdr[:], xd_ps[:], AF.Copy, scale=rs2[:])
                    nc.vector.tensor_tensor(
                        xnew[:, dc * 512:(dc + 1) * 512],
                        x1[:, dc * 512:(dc + 1) * 512], xdr[:], op=ALU.add,
                    )
                if l + 1 < n_layers:
                    tile_stats(xnew[:], ssq_n, mx_n, i, scale_bc_next)
                    if i == 3:
                        stats_chain(ssq_n, mx_n, c1_n, rs_n, 0, 4)
                    elif i == n_tok_tiles - 1:
                        stats_chain(ssq_n, mx_n, c1_n, rs_n, 4, n_tok_tiles)
                nc.scalar.dma_start(xdst[i * 128:(i + 1) * 128, :], xnew[:])
                std[i] = None

            # software pipeline with one-tile lookahead on q/qT:
            # q0 q1 | mm0 q2 | mm1 q3 dn0 | mm2 q4 dn1 | ... | mm7 dn6 | dn7
            emit_q(0)
            emit_q(1)
            for i in range(n_tok_tiles):
                emit_mm(i)
                if i + 2 < n_tok_tiles:
                    emit_q(i + 2)
                if i >= 1:
                    emit_down(i - 1)
                if l == 0 and i == 5:
                    emit_tail_wmax()
            emit_down(n_tok_tiles - 1)

            xsrc = xdst
            if l + 1 < n_layers:
                wg8 = dram.tile([128, NDK, F], dt.float8e4, tag="wg8")
                wu8 = dram.tile([128, NDK, F], dt.float8e4, tag="wu8")
                wd8 = dram.tile([128, NFT, D], dt.float8e4, tag="wd8")
                w8_next = (wg8, wu8, wd8)
                tern_layer(
                    l + 1,
                    lambda ch, dk: wg8[:, dk, ch * 1024:(ch + 1) * 1024],
                    lambda ch, dk: wu8[:, dk, ch * 1024:(ch + 1) * 1024],
                    lambda ft: wd8[:, ft, :],
                    via_dram=True,
                )

    _split_excess_waits(nc)
    return nc


_nc_cache = {}


def _get_nc(key):
    if key not in _nc_cache:
        _nc_cache[key] = build(*key)
    return _nc_cache[key]


def _repack(w, n_layers):
    # [L, D, F] -> [L, F//1024, D//128, 128, 1024] so each (ch, dk) tile is
    # contiguous; dk == core index for the per-core max slices.
    return np.ascontiguousarray(
        w.reshape(n_layers, D // 128, 128, NCH, 1024).transpose(0, 3, 1, 2, 4)
    )


def _make_in_maps(x, rs, wg, wu, wd, n_cores=NCORES):
    n_layers = rs.shape[0]
    wg_r = _repack(wg, n_layers)
    wu_r = _repack(wu, n_layers)
    nsl = NDK // n_cores  # dk-slabs per core for slg/slu
    dsl = F // n_cores
    in_maps = []
    for c in range(n_cores):
        slg = wg_r[:, :, c * nsl:(c + 1) * nsl].reshape(n_layers, -1, 128, 1024)
        slu = wu_r[:, :, c * nsl:(c + 1) * nsl].reshape(n_layers, -1, 128, 1024)
        sld = wd[:, c * dsl:(c + 1) * dsl, :].reshape(n_layers, -1, 128, 1024)
        in_maps.append({
            "x": x[c],
            "rs": rs,
            "wg": wg_r,
            "wu": wu_r,
            "wd": wd,
            "slg": np.ascontiguousarray(slg),
            "slu": np.ascontiguousarray(slu),
            "sld": np.ascontiguousarray(sld),
        })
    return in_maps


def kernel(x, rms_scale, W_g, W_u, W_d):
    """Full-input entry point: shard over batch, run 8-core SPMD, gather."""
    x = np.ascontiguousarray(np.asarray(x, dtype=np.float32))
    rs = np.ascontiguousarray(np.asarray(rms_scale, dtype=np.float32))
    wg = np.ascontiguousarray(np.asarray(W_g, dtype=np.float32))
    wu = np.ascontiguousarray(np.asarray(W_u, dtype=np.float32))
    wd = np.ascontiguousarray(np.asarray(W_d, dtype=np.float32))
    B, S, Dx = x.shape
    assert (B, S, Dx) == (NCORES, TOK, D), (B, S, Dx)
    is_ones = bool(np.all(rs == 1.0))
    nc = _get_nc((is_ones,))
    in_maps = _make_in_maps(x, rs, wg, wu, wd)
    res = run_bass_kernel_spmd(nc, in_maps, list(range(NCORES)))
    return np.stack([res.results[c]["out"] for c in range(NCORES)], axis=0)


# revision 16
# speedup vs baseline: 1.1602x; 1.1501x over previous
"""Trainium2 Bass kernel for nn_CascadeTransformerMM (4-layer ternary-GLU cascade).

Math (per layer, per token row):
  h   = rms_scale * x * rsqrt(mean(x^2) + 1e-6)
  s   = clip(127/(max|h| + 1e-5), 1e-3, 1e3);  q = round(s*h)      (ints in [-127,127])
  Wt  = clip(round(W * 127/(max|W| + 1e-5)), -1, 1)                 (ternary {-1,0,1})
  u   = (q @ Wg_t)/s ; v = (q @ Wu_t)/s ; g = silu(u)*v
  s2  = clip(127/(max|g| + 1e-5), 1e-3, 1e3); gq = round(s2*g)
  x  += (gq @ Wd_t)/s2

Distribution: pure data-parallel over the batch dim (8 batches -> 8 cores),
weights replicated per core. Per-matrix |W|max is computed cooperatively:
each core reduces a 1/8 row-slice, then a tiny AllReduce(max) shares the 12
scalars (layer 0's three matrices allreduced first so its ternarize can
start early). All matmuls run on the PE array with bf16 activations
(integers <= 127, exact) x fp8 ternary weights (exact), fp32 PSUM
accumulation -> the heavy compute is bit-exact integer arithmetic.

Schedule: per-token-tile software pipeline with one-tile lookahead --
q/qT production for tile i+2 and down-proj for tile i-1 are emitted
around the up-proj matmul block of tile i, so the serial per-tile tail
(s2 chain -> gq round -> gqT transpose) hides under the next tile's
matmuls and the PE stream stays dense (keeps the HAM clock-gate warm).
Weights live in per-column-chunk SBUF tiles so layer-0 ternarize and
layer-boundary refills unblock the matmul stream progressively.
Ternarize runs pass 1 (scale+round-to-int32) on DVE and pass 2 (clip to
{-1,0,1} as fp8) on the otherwise-idle GPSIMD (alternating with ACT
Sign for layer 0, where latency matters)."""

import os
import sys

for _p in ("/opt/trn_rl_repo", "/root/.axon_site/_ro/trn_rl_repo"):
    if os.path.isdir(_p) and _p not in sys.path:
        sys.path.insert(0, _p)

import numpy as np
from contextlib import ExitStack

import concourse.bass as bass
import concourse.bass_isa as bass_isa
import concourse.mybir as mybir
import concourse.tile as tile
from concourse.bass_utils import run_bass_kernel_spmd

dt = mybir.dt
AF = mybir.ActivationFunctionType
ALU = mybir.AluOpType

MAGIC = float(1.5 * 2**23)  # fp32 round-to-nearest-even magic constant
D = 1024
F = 4096
L = 4
NCORES = 8
TOK = 1024  # tokens per core (one batch of S=1024)

NDK = D // 128   # 8 contraction tiles for up-proj
NFT = F // 128   # 32 contraction tiles for down-proj
NFC = F // 512   # 8 free-dim chunks for up-proj
NCH = F // 1024  # 4 column-chunks in the repacked wg/wu layout
NTT = TOK // 128  # 8 token tiles


def _split_excess_waits(nc, max_waits: int = 1) -> int:
    """walrus in this container rejects >1 sync-wait per instruction; split
    extras into standalone event-semaphore waits on the same engine (same-
    engine program order makes this semantically identical)."""
    n = 0
    for func in nc.m.functions:
        for block in func.blocks:
            changed = False
            out = []
            for inst in block.instructions:
                si = getattr(inst, "sync_info", None)
                if si is not None and si.on_wait and len(si.on_wait) > max_waits:
                    waits = list(si.on_wait)
                    for j, w in enumerate(waits[max_waits:]):
                        out.append(
                            mybir.InstEventSemaphore(
                                name=f"{inst.name}-xw{j}",
                                engine=inst.engine,
                                ins=[],
                                outs=[],
                                sync_info=mybir.SyncInfo(on_wait=[w], on_update=[]),
                            )
                        )
                        n += 1
                    inst.sync_info = mybir.SyncInfo(
                        on_wait=waits[:max_waits], on_update=list(si.on_update)
                    )
                    changed = True
                out.append(inst)
            if changed:
                block.instructions = out
    return n


def build(is_ones: bool = True, n_cores: int = NCORES, n_tok_tiles: int = NTT,
          n_layers: int = L) -> bass.Bass:
    nc = bass.Bass(num_devices=n_cores)
    tok = n_tok_tiles * 128

    x_ext = nc.declare_dram_parameter("x", [tok, D], dt.float32, isOutput=False)
    rs_ext = nc.declare_dram_parameter("rs", [n_layers, D], dt.float32, isOutput=False)
    # wg/wu repacked host-side to [L, F//1024, NDK, 128, 1024] so every
    # [128, 1024] weight tile is one contiguous 512 KB DMA; wd's row-slabs
    # are naturally contiguous.
    wg_ext = nc.declare_dram_parameter("wg", [n_layers, NCH, NDK, 128, 1024], dt.float32, isOutput=False)
    wu_ext = nc.declare_dram_parameter("wu", [n_layers, NCH, NDK, 128, 1024], dt.float32, isOutput=False)
    wd_ext = nc.declare_dram_parameter("wd", [n_layers, F, D], dt.float32, isOutput=False)
    # per-core row-slices of each matrix for the cooperative |W|max,
    # as [L, nun, 128, 1024] contiguous units
    nun = (D // n_cores) * F // (128 * 1024)
    slg_ext = nc.declare_dram_parameter("slg", [n_layers, nun, 128, 1024], dt.float32, isOutput=False)
    slu_ext = nc.declare_dram_parameter("slu", [n_layers, nun, 128, 1024], dt.float32, isOutput=False)
    sld_ext = nc.declare_dram_parameter("sld", [n_layers, nun, 128, 1024], dt.float32, isOutput=False)
    out_ext = nc.declare_dram_parameter("out", [tok, D], dt.float32, isOutput=True)

    mx_locs = [nc.dram_tensor(f"mx{l}_loc", [1, 16], dt.float32) for l in range(n_layers)]
    mx_globs = [nc.dram_tensor(f"mx{l}_glob", [1, 16], dt.float32) for l in range(n_layers)]
    # AllGather outputs for cooperative ternarize of layers >= 1:
    # [core, 128, 12288] fp8 where cols 0:4096 = wg (ch-major), 4096:8192 = wu,
    # 8192:12288 = wd (4 ft units)
    gall = [None] + [
        nc.dram_tensor(f"gall{l}", [n_cores, 128, 12288], dt.float8e4, addr_space="Shared")
        for l in range(1, n_layers)
    ]

    with tile.TileContext(nc) as tc, ExitStack() as ctx:
        P = ctx.enter_context
        wch = P(tc.tile_pool(name="wch", bufs=2 * NCH))   # per-ch wg/wu fp8 tiles
        wdp = P(tc.tile_pool(name="wdp", bufs=1))         # wd fp8 tile
        wstream = P(tc.tile_pool(name="wstream", bufs=2))
        wi32 = P(tc.tile_pool(name="wi32", bufs=2))
        xpool = P(tc.tile_pool(name="x1", bufs=5))
        t1pool = P(tc.tile_pool(name="t1", bufs=1))
        s4k = P(tc.tile_pool(name="s4k", bufs=2))         # xnew scratch
        qpool = P(tc.tile_pool(name="q", bufs=2))
        qtpool = P(tc.tile_pool(name="qt", bufs=2))
        silupool = P(tc.tile_pool(name="silu", bufs=2))   # [128,512] silu chunks
        scrpool = P(tc.tile_pool(name="scr", bufs=1))     # square scratch
        gpool = P(tc.tile_pool(name="g", bufs=1))
        gqpool = P(tc.tile_pool(name="gq", bufs=1))
        gqtpool = P(tc.tile_pool(name="gqt", bufs=2))
        w8s = P(tc.tile_pool(name="w8s", bufs=1))
        batch = P(tc.tile_pool(name="batch", bufs=2))     # [128, ntt] per-layer stats
        btmp = P(tc.tile_pool(name="btmp", bufs=1))       # stats chain temps
        sc = P(tc.tile_pool(name="sc", bufs=4))           # [128, small] scalars
        xdrpool = P(tc.tile_pool(name="xdr", bufs=1))
        const = P(tc.tile_pool(name="const", bufs=1))
        bcpool = P(tc.tile_pool(name="bc", bufs=2))       # rms_scale broadcast (general path)
        dram = P(tc.tile_pool(name="dram", bufs=2, space="DRAM"))
        ps_up = P(tc.tile_pool(name="psup", bufs=4, space="PSUM"))
        ps_dn = P(tc.tile_pool(name="psdn", bufs=4, space="PSUM"))

        # ---------- constants ----------
        mag = const.tile([128, 1], dt.float32, tag="mag")
        nc.gpsimd.memset(mag[:], MAGIC)
        ones1 = const.tile([1, 128], dt.float32, tag="ones1")
        nc.gpsimd.memset(ones1[:], 1.0)

        def pe_broadcast(dst, src_row, n):
            """broadcast src_row [1, n] to dst [128, n] via PE outer product."""
            for h in range(0, n, 512):
                w = min(512, n - h)
                bc_ps = ps_up.tile([128, 512], dt.float32, tag="ups")
                nc.tensor.matmul(bc_ps[:, 0:w], ones1[:], src_row[:, h:h + w],
                                 start=True, stop=True)
                nc.scalar.activation(dst[:, h:h + w], bc_ps[:, 0:w], AF.Copy)

        # ---------- cooperative per-matrix |W|max ----------
        # Layer 0's three matrices are reduced + allreduced first so its
        # ternarize can start while layers 1-3 slices still stream.
        wmax_cols = const.tile([128, 16], dt.float32, tag="wmaxc")
        nc.gpsimd.memset(wmax_cols[:], 0.0)

        def slab_max(l, ext, idx, eng):
            part = sc.tile([128, nun], dt.float32, tag="wmaxpart")
            for u in range(nun):
                wt = wstream.tile([128, 1024], dt.float32, tag="wstream")
                eng.dma_start(wt[:], ext[l, u])
                nc.vector.tensor_reduce(
                    part[:, u:u + 1], wt[:], axis=mybir.AxisListType.X,
                    op=ALU.max, apply_absolute_value=True,
                )
            nc.vector.tensor_reduce(
                wmax_cols[:, idx:idx + 1], part[:, 0:nun],
                axis=mybir.AxisListType.X, op=ALU.max,
                apply_absolute_value=False,
            )

        wscl = []
        for _l in range(n_layers):
            wsc_t = const.tile([128, 16], dt.float32, tag=f"wsc{_l}")
            wscl.append(wsc_t)
        grow = const.tile([1, 16], dt.float32, tag="grow")
        mrow = const.tile([1, 16], dt.float32, tag="mrow")
        nc.gpsimd.memset(mrow[:], 0.0)
        wsc_row = const.tile([1, 16], dt.float32, tag="wscrow")

        def finish_wsc(cols, loc, glob, wsc):
            """partition-reduce wmax cols, allreduce across cores, build
            wsc[:, cols] = 127/(m+1e-5) broadcast to all partitions."""
            a, b = cols
            nc.gpsimd.tensor_reduce(
                mrow[:, a:b], wmax_cols[:, a:b], axis=mybir.AxisListType.C, op=ALU.max
            )
            nc.scalar.dma_start(loc[:], mrow[:])
            nc.gpsimd.collective_compute(
                "AllReduce",
                ALU.max,
                replica_groups=[list(range(n_cores))],
                ins=[loc[:].opt()],
                outs=[glob[:].opt()],
            )
            nc.scalar.dma_start(grow[:, a:b], glob[:, a:b])
            nc.vector.tensor_scalar(wsc_row[:, a:b], grow[:, a:b], 1e-5, None, op0=ALU.add)
            nc.vector.reciprocal(wsc_row[:, a:b], wsc_row[:, a:b])
            nc.vector.tensor_scalar(wsc_row[:, a:b], wsc_row[:, a:b], 127.0, None, op0=ALU.mult)
            pe_broadcast(wsc, wsc_row, 16)

        for mi, ext in enumerate((slg_ext, slu_ext, sld_ext)):
            slab_max(0, ext, mi, nc.scalar)
        finish_wsc((0, 3), mx_locs[0], mx_globs[0], wscl[0])

        # ---------- ternarize ----------
        def tern_unit(src, dst_ap, idx, pass2_eng):
            """src: [128, 1024] fp32 AP; dst_ap: fp8 [128, 1024]-shaped."""
            wt = wstream.tile([128, 1024], dt.float32, tag="wstream")
            nc.sync.dma_start(wt[:], src)
            r32 = wi32.tile([128, 1024], dt.int32, tag="wi32")
            wsc = wscl[idx // 3]
            nc.vector.tensor_scalar(
                r32[:], wt[:], wsc[:, idx:idx + 1], None, op0=ALU.mult
            )
            if pass2_eng == "act":
                # sign(n) == clip(n, -1, 1) for integer n
                nc.scalar.activation(dst_ap, r32[:], AF.Sign)
            else:
                nc.gpsimd.tensor_scalar(dst_ap, r32[:], 1, -1, op0=ALU.min, op1=ALU.max)

        def tern_layer(l, dst_g, dst_u, dst_d, via_dram, split_pass2=False):
            """dst_g/dst_u: fn(ch, dk) -> fp8 [128,1024] AP; dst_d: fn(ft) -> AP."""
            cnt = [0]

            def unit(src, dst_ap, idx):
                eng = "act" if (split_pass2 and cnt[0] % 2 == 0) else "gp"
                cnt[0] += 1
                if via_dram:
                    stg = w8s.tile([128, 1024], dt.float8e4, tag="w8s")
                    tern_unit(src, stg[:], idx, eng)
                    nc.sync.dma_start(dst_ap, stg[:])
                else:
                    tern_unit(src, dst_ap, idx, eng)

            for ch in range(NCH):
                for dk in range(NDK):
                    unit(wg_ext[l, ch, dk], dst_g(ch, dk), 3 * l)
                    unit(wu_ext[l, ch, dk], dst_u(ch, dk), 3 * l + 1)
            for ft in range(NFT):
                unit(wd_ext[l, ft * 128:(ft + 1) * 128, :], dst_d(ft), 3 * l + 2)

        # ---------- cooperative weight prefetch (layers >= 1) ----------
        # Each core reduces + ternarizes only its 1/8 shard (the same
        # host-staged slices used for the |W|max pass), then one fp8
        # AllGather per layer shares the ternary weights; the gather hides
        # under the previous layer's compute.
        def coop_prefetch(lp):
            for mi, ext in enumerate((slg_ext, slu_ext, sld_ext)):
                slab_max(lp, ext, 3 * lp + mi, nc.sync)
            finish_wsc((3 * lp, 3 * lp + 3), mx_locs[lp], mx_globs[lp], wscl[lp])
            loc_all = dram.tile([128, 12288], dt.float8e4, tag="locall")
            for mi, ext in enumerate((slg_ext, slu_ext, sld_ext)):
                for u in range(nun):
                    stg = w8s.tile([128, 1024], dt.float8e4, tag="w8s")
                    tern_unit(ext[lp, u], stg[:], 3 * lp + mi, "gp")
                    nc.sync.dma_start(
                        loc_all[:, mi * 4096 + u * 1024:mi * 4096 + (u + 1) * 1024],
                        stg[:],
                    )
            nc.gpsimd.collective_compute(
                "AllGather",
                ALU.bypass,
                replica_groups=[list(range(n_cores))],
                ins=[loc_all[:].opt()],
                outs=[gall[lp][:].opt()],
            )

        # ---------- per-layer stats ----------
        # ssq/mx filled per token tile; the scalar chain runs on column
        # groups [0:4] and [4:8] as soon as those tiles' stats exist.
        def stats_chain(ssq_all, mx_all, c1_all, rs_all, a, b):
            k = b - a
            ms_t = btmp.tile([128, n_tok_tiles], dt.float32, tag="ms")
            rt_t = btmp.tile([128, n_tok_tiles], dt.float32, tag="rt")
            rstd_t = btmp.tile([128, n_tok_tiles], dt.float32, tag="rstd")
            nwt_t = btmp.tile([128, n_tok_tiles], dt.float32, tag="nwt")
            maxh_t = btmp.tile([128, n_tok_tiles], dt.float32, tag="maxh")
            sr_t = btmp.tile([128, n_tok_tiles], dt.float32, tag="sr")
            s_t = btmp.tile([128, n_tok_tiles], dt.float32, tag="s_")
            ms, rt, rstd = ms_t[:, 0:k], rt_t[:, 0:k], rstd_t[:, 0:k]
            nwt, maxh, sr, s_ = nwt_t[:, 0:k], maxh_t[:, 0:k], sr_t[:, 0:k], s_t[:, 0:k]
            nc.vector.tensor_scalar(ms, ssq_all[:, a:b], 1.0 / D, 1e-6, op0=ALU.mult, op1=ALU.add)
            nc.scalar.activation(rt, ms, AF.Sqrt)
            nc.vector.reciprocal(rstd, rt)
            # one Newton step: rstd *= 1.5 - 0.5*ms*rstd^2  (fixes the ~7e-6
            # Sqrt-LUT error that quantization tie-flips amplify layer by layer)
            nc.vector.tensor_tensor(nwt, rstd, rstd, op=ALU.mult)
            nc.vector.tensor_tensor(nwt, nwt, ms, op=ALU.mult)
            nc.vector.tensor_scalar(nwt, nwt, -0.5, 1.5, op0=ALU.mult, op1=ALU.add)
            nc.vector.tensor_tensor(rstd, rstd, nwt, op=ALU.mult)
            nc.vector.tensor_tensor(maxh, mx_all[:, a:b], rstd, op=ALU.mult)
            nc.vector.tensor_scalar(maxh, maxh, 1e-5, None, op0=ALU.add)
            nc.vector.reciprocal(sr, maxh)
            nc.vector.tensor_scalar(s_, sr, 127.0, 1e3, op0=ALU.mult, op1=ALU.min)
            nc.vector.tensor_scalar(s_, s_, 1e-3, None, op0=ALU.max)
            nc.vector.tensor_tensor(c1_all[:, a:b], s_, rstd, op=ALU.mult)
            nc.vector.reciprocal(rs_all[:, a:b], s_)

        def tile_stats(src_ap, ssq_all, mx_all, i, scale_bc):
            """ssq + scaled abs-max for token tile i of the NEXT layer's input."""
            scr = scrpool.tile([128, D], dt.float32, tag="scr")
            nc.scalar.activation(scr[:], src_ap, AF.Square, accum_out=ssq_all[:, i:i + 1])
            if is_ones:
                nc.vector.tensor_reduce(
                    mx_all[:, i:i + 1], src_ap, axis=mybir.AxisListType.X,
                    op=ALU.max, apply_absolute_value=True,
                )
            else:
                nc.vector.tensor_tensor(scr[:], src_ap, scale_bc[:], op=ALU.mult)
                nc.vector.tensor_reduce(
                    mx_all[:, i:i + 1], scr[:], axis=mybir.AxisListType.X,
                    op=ALU.max, apply_absolute_value=True,
                )

        # ---------- layers ----------
        xsrc = x_ext
        prev_stats = None  # (ssq, mx, c1, rs) for current layer, from fused pass

        for l in range(n_layers):
            wg_c, wu_c = [], []
            for _ch in range(NCH):
                wgc_t = wch.tile([128, NDK, 1024], dt.float8e4, tag="wch")
                wg_c.append(wgc_t)
            for _ch in range(NCH):
                wuc_t = wch.tile([128, NDK, 1024], dt.float8e4, tag="wch")
                wu_c.append(wuc_t)
            wd_t = wdp.tile([128, NFT, D], dt.float8e4, tag="wdp")
            if l == 0:
                tern_layer(
                    0,
                    lambda ch, dk: wg_c[ch][:, dk, :],
                    lambda ch, dk: wu_c[ch][:, dk, :],
                    lambda ft: wd_t[:, ft, :],
                    via_dram=False,
                    split_pass2=True,
                )
                coop_prefetch(1)
            else:
                ga = gall[l]
                # fill in consumption order: wg/wu chunk-columns, then wd
                for ch in range(NCH):
                    nc.sync.dma_start(
                        wg_c[ch][:],
                        ga[:, :, ch * 1024:(ch + 1) * 1024].transpose([1, 0, 2]))
                    nc.sync.dma_start(
                        wu_c[ch][:],
                        ga[:, :, 4096 + ch * 1024:4096 + (ch + 1) * 1024].transpose([1, 0, 2]))
                for c in range(n_cores):
                    nc.sync.dma_start(wd_t[:, 4 * c:4 * c + 4, :], ga[c, :, 8192:12288])
                if l + 1 < n_layers:
                    coop_prefetch(l + 1)

            scale_bc = None
            scale_bc_next = None
            if not is_ones:
                rs_row = const.tile([1, D], dt.float32, tag=f"rsrow{l}")
                nc.sync.dma_start(rs_row[:], rs_ext[l:l + 1, :])
                scale_bc = bcpool.tile([128, D], dt.float32, tag="bc")
                pe_broadcast(scale_bc, rs_row[:], D)
                if l + 1 < n_layers:
                    rs_row_n = const.tile([1, D], dt.float32, tag=f"rsrown{l}")
                    nc.sync.dma_start(rs_row_n[:], rs_ext[l + 1:l + 2, :])
                    scale_bc_next = bcpool.tile([128, D], dt.float32, tag="bcn")
                    pe_broadcast(scale_bc_next, rs_row_n[:], D)

            if l == 0:
                # phase A for layer 0 only: standalone stats pass
                ssq_all = batch.tile([128, n_tok_tiles], dt.float32, tag="ssq")
                mx_all = batch.tile([128, n_tok_tiles], dt.float32, tag="mx")
                c1_all = batch.tile([128, n_tok_tiles], dt.float32, tag="c1")
                rs_all = batch.tile([128, n_tok_tiles], dt.float32, tag="rs_all")
                for i in range(n_tok_tiles):
                    xa = xpool.tile([128, D], dt.float32, tag="x1")
                    nc.scalar.dma_start(xa[:], xsrc[i * 128:(i + 1) * 128, :])
                    tile_stats(xa[:], ssq_all, mx_all, i, scale_bc)
                    if i == 3:
                        stats_chain(ssq_all, mx_all, c1_all, rs_all, 0, 4)
                    elif i == n_tok_tiles - 1:
                        stats_chain(ssq_all, mx_all, c1_all, rs_all, 4, n_tok_tiles)
            else:
                ssq_all, mx_all, c1_all, rs_all = prev_stats

            if l + 1 < n_layers:
                ssq_n = batch.tile([128, n_tok_tiles], dt.float32, tag="ssq")
                mx_n = batch.tile([128, n_tok_tiles], dt.float32, tag="mx")
                c1_n = batch.tile([128, n_tok_tiles], dt.float32, tag="c1")
                rs_n = batch.tile([128, n_tok_tiles], dt.float32, tag="rs_all")
                prev_stats = (ssq_n, mx_n, c1_n, rs_n)

            xdst = out_ext if l == n_layers - 1 else dram.tile([tok, D], dt.float32, tag="xbuf")

            # per-tile state
            stq = [None] * n_tok_tiles   # (x1, qT) from emit_q
            std = [None] * n_tok_tiles   # (x1, gqT, rs2) from emit_mm

            def emit_q(i):
                """load x tile, quantize, transpose -- one tile ahead of the MMs."""
                x1 = xpool.tile([128, D], dt.float32, tag="x1")
                nc.scalar.dma_start(x1[:], xsrc[i * 128:(i + 1) * 128, :])
                t1 = t1pool.tile([128, D], dt.float32, tag="t1")
                # q = round(c1 * h') via magic add (ACT) / subtract (DVE), out bf16
                if is_ones:
                    nc.scalar.activation(t1[:], x1[:], AF.Identity,
                                         scale=c1_all[:, i:i + 1], bias=mag[:])
                else:
                    nc.vector.tensor_tensor(t1[:], x1[:], scale_bc[:], op=ALU.mult)
                    nc.scalar.activation(t1[:], t1[:], AF.Identity,
                                         scale=c1_all[:, i:i + 1], bias=mag[:])
                q = qpool.tile([128, D], dt.bfloat16, tag="q")
                nc.vector.tensor_scalar(q[:], t1[:], MAGIC, None, op0=ALU.subtract)
                qT = qtpool.tile([128, NDK, 128], dt.bfloat16, tag="qt")
                nc.scalar.dma_start_transpose(qT[:], q[:])
                stq[i] = (x1, qT)

            def emit_mm(i):
                """up-proj matmuls + GLU + act-quant + gqT for tile i."""
                x1, qT = stq[i]
                g = gpool.tile([128, F], dt.float32, tag="g")
                gm8 = sc.tile([128, NFC], dt.float32, tag="gm8")
                for f in range(NFC):
                    ch = f // 2
                    fo = (f % 2) * 512
                    u_ps = ps_up.tile([128, 512], dt.float32, tag="ups")
                    v_ps = ps_up.tile([128, 512], dt.float32, tag="ups")
                    for dk in range(NDK):
                        nc.tensor.matmul(
                            u_ps[:], qT[:, dk, :], wg_c[ch][:, dk, fo:fo + 512],
                            start=(dk == 0), stop=(dk == NDK - 1),
                        )
                        nc.tensor.matmul(
                            v_ps[:], qT[:, dk, :], wu_c[ch][:, dk, fo:fo + 512],
                            start=(dk == 0), stop=(dk == NDK - 1),
                        )
                    su = silupool.tile([128, 512], dt.float32, tag="silu")
                    nc.scalar.activation(su[:], u_ps[:], AF.Silu,
                                         scale=rs_all[:, i:i + 1])
                    nc.vector.tensor_tensor(
                        g[:, f * 512:(f + 1) * 512], su[:], v_ps[:], op=ALU.mult
                    )
                    nc.vector.tensor_reduce(
                        gm8[:, f:f + 1], g[:, f * 512:(f + 1) * 512],
                        axis=mybir.AxisListType.X, op=ALU.max, apply_absolute_value=True,
                    )
                # s2 = clip(127/(max|g|/s + 1e-5)); c2 = s2/s ; rs2 = 1/s2
                gmx = sc.tile([128, 1], dt.float32, tag="gmx")
                nc.vector.tensor_reduce(
                    gmx[:], gm8[:], axis=mybir.AxisListType.X, op=ALU.max,
                    apply_absolute_value=False,
                )
                nc.vector.tensor_tensor(gmx[:], gmx[:], rs_all[:, i:i + 1], op=ALU.mult)
                nc.vector.tensor_scalar(gmx[:], gmx[:], 1e-5, None, op0=ALU.add)
                s2r = sc.tile([128, 1], dt.float32, tag="s2r")
                nc.vector.reciprocal(s2r[:], gmx[:])
                s2 = sc.tile([128, 1], dt.float32, tag="s2")
                nc.vector.tensor_scalar(s2[:], s2r[:], 127.0, 1e3, op0=ALU.mult, op1=ALU.min)
                nc.vector.tensor_scalar(s2[:], s2[:], 1e-3, None, op0=ALU.max)
                c2 = sc.tile([128, 1], dt.float32, tag="c2")
                nc.vector.tensor_tensor(c2[:], s2[:], rs_all[:, i:i + 1], op=ALU.mult)
                rs2 = sc.tile([128, 1], dt.float32, tag="rs2")
                nc.vector.reciprocal(rs2[:], s2[:])
                # gq = round(c2*g) in two halves (ACT magic pass in place, DVE
                # subtract to bf16, transpose) so the serial tail is half as long
                gq = gqpool.tile([128, F], dt.bfloat16, tag="gq")
                gqT = gqtpool.tile([128, NFT, 128], dt.bfloat16, tag="gqt")
                for h in range(2):
                    hs = h * (F // 2)
                    he = hs + F // 2
                    nc.scalar.activation(g[:, hs:he], g[:, hs:he], AF.Identity,
                                         scale=c2[:], bias=mag[:])
                    nc.vector.tensor_scalar(gq[:, hs:he], g[:, hs:he], MAGIC, None,
                                            op0=ALU.subtract)
                    nc.scalar.dma_start_transpose(
                        gqT[:, h * (NFT // 2):(h + 1) * (NFT // 2), :], gq[:, hs:he])
                std[i] = (x1, gqT, rs2)
                stq[i] = None

            def emit_down(i):
                x1, gqT, rs2 = std[i]
                xd_ps0 = ps_dn.tile([128, 512], dt.float32, tag="dps")
                xd_ps1 = ps_dn.tile([128, 512], dt.float32, tag="dps")
                for ft in range(NFT):
                    nc.tensor.matmul(
                        xd_ps0[:], gqT[:, ft, :], wd_t[:, ft, 0:512],
                        start=(ft == 0), stop=(ft == NFT - 1),
                    )
                    nc.tensor.matmul(
                        xd_ps1[:], gqT[:, ft, :], wd_t[:, ft, 512:1024],
                        start=(ft == 0), stop=(ft == NFT - 1),
                    )
                xnew = s4k.tile([128, D], dt.float32, tag="s4k")
                for dc, xd_ps in ((0, xd_ps0), (1, xd_ps1)):
                    xdr = xdrpool.tile([128, 512], dt.float32, tag="xdr")
                    nc.scalar.activation(xdr[:], xd_ps[:], AF.Copy, scale=rs2[:])
                    nc.vector.tensor_tensor(
                        xnew[:, dc * 512:(dc + 1) * 512],
                        x1[:, dc * 512:(dc + 1) * 512], xdr[:], op=ALU.add,
                    )
                if l + 1 < n_layers:
                    tile_stats(xnew[:], ssq_n, mx_n, i, scale_bc_next)
                    if i == 3:
                        stats_chain(ssq_n, mx_n, c1_n, rs_n, 0, 4)
                    elif i == n_tok_tiles - 1:
                        stats_chain(ssq_n, mx_n, c1_n, rs_n, 4, n_tok_tiles)
                nc.scalar.dma_start(xdst[i * 128:(i + 1) * 128, :], xnew[:])
                std[i] = None

            # software pipeline with one-tile lookahead on q/qT:
            # q0 q1 | mm0 q2 | mm1 q3 dn0 | mm2 q4 dn1 | ... | mm7 dn6 | dn7
            emit_q(0)
            emit_q(1)
            for i in range(n_tok_tiles):
                emit_mm(i)
                if i + 2 < n_tok_tiles:
                    emit_q(i + 2)
                if i >= 1:
                    emit_down(i - 1)
            emit_down(n_tok_tiles - 1)

            xsrc = xdst

    _split_excess_waits(nc)
    return nc


_nc_cache = {}


def _get_nc(key):
    if key not in _nc_cache:
        _nc_cache[key] = build(*key)
    return _nc_cache[key]


def _repack(w, n_layers):
    # [L, D, F] -> [L, F//1024, D//128, 128, 1024] so each (ch, dk) tile is
    # contiguous; dk == core index for the per-core max slices.
    return np.ascontiguousarray(
        w.reshape(n_layers, D // 128, 128, NCH, 1024).transpose(0, 3, 1, 2, 4)
    )


def _make_in_maps(x, rs, wg, wu, wd, n_cores=NCORES):
    n_layers = rs.shape[0]
    wg_r = _repack(wg, n_layers)
    wu_r = _repack(wu, n_layers)
    nsl = NDK // n_cores  # dk-slabs per core for slg/slu
    dsl = F // n_cores
    in_maps = []
    for c in range(n_cores):
        slg = wg_r[:, :, c * nsl:(c + 1) * nsl].reshape(n_layers, -1, 128, 1024)
        slu = wu_r[:, :, c * nsl:(c + 1) * nsl].reshape(n_layers, -1, 128, 1024)
        sld = wd[:, c * dsl:(c + 1) * dsl, :].reshape(n_layers, -1, 128, 1024)
        in_maps.append({
            "x": x[c],
            "rs": rs,
            "wg": wg_r,
            "wu": wu_r,
            "wd": wd,
            "slg": np.ascontiguousarray(slg),
            "slu": np.ascontiguousarray(slu),
            "sld": np.ascontiguousarray(sld),
        })
    return in_maps


def kernel(x, rms_scale, W_g, W_u, W_d):
    """Full-input entry point: shard over batch, run 8-core SPMD, gather."""
    x = np.ascontiguousarray(np.asarray(x, dtype=np.float32))
    rs = np.ascontiguousarray(np.asarray(rms_scale, dtype=np.float32))
    wg = np.ascontiguousarray(np.asarray(W_g, dtype=np.float32))
    wu = np.ascontiguousarray(np.asarray(W_u, dtype=np.float32))
    wd = np.ascontiguousarray(np.asarray(W_d, dtype=np.float32))
    B, S, Dx = x.shape
    assert (B, S, Dx) == (NCORES, TOK, D), (B, S, Dx)
    is_ones = bool(np.all(rs == 1.0))
    nc = _get_nc((is_ones,))
    in_maps = _make_in_maps(x, rs, wg, wu, wd)
    res = run_bass_kernel_spmd(nc, in_maps, list(range(NCORES)))
    return np.stack([res.results[c]["out"] for c in range(NCORES)], axis=0)


# revision 17
# speedup vs baseline: 1.1926x; 1.0280x over previous
"""Trainium2 Bass kernel for nn_CascadeTransformerMM (4-layer ternary-GLU cascade).

Math (per layer, per token row):
  h   = rms_scale * x * rsqrt(mean(x^2) + 1e-6)
  s   = clip(127/(max|h| + 1e-5), 1e-3, 1e3);  q = round(s*h)      (ints in [-127,127])
  Wt  = clip(round(W * 127/(max|W| + 1e-5)), -1, 1)                 (ternary {-1,0,1})
  u   = (q @ Wg_t)/s ; v = (q @ Wu_t)/s ; g = silu(u)*v
  s2  = clip(127/(max|g| + 1e-5), 1e-3, 1e3); gq = round(s2*g)
  x  += (gq @ Wd_t)/s2

Distribution: pure data-parallel over the batch dim (8 batches -> 8 cores),
weights replicated per core. Per-matrix |W|max is computed cooperatively:
each core reduces a 1/8 row-slice, then a tiny AllReduce(max) shares the 12
scalars (layer 0's three matrices allreduced first so its ternarize can
start early). All matmuls run on the PE array with bf16 activations
(integers <= 127, exact) x fp8 ternary weights (exact), fp32 PSUM
accumulation -> the heavy compute is bit-exact integer arithmetic.

Schedule: per-token-tile software pipeline with one-tile lookahead --
q/qT production for tile i+2 and down-proj for tile i-1 are emitted
around the up-proj matmul block of tile i, so the serial per-tile tail
(s2 chain -> gq round -> gqT transpose) hides under the next tile's
matmuls and the PE stream stays dense (keeps the HAM clock-gate warm).
Weights live in per-column-chunk SBUF tiles so layer-0 ternarize and
layer-boundary refills unblock the matmul stream progressively.
Ternarize runs pass 1 (scale+round-to-int32) on DVE and pass 2 (clip to
{-1,0,1} as fp8) on the otherwise-idle GPSIMD (alternating with ACT
Sign for layer 0, where latency matters)."""

import os
import sys

for _p in ("/opt/trn_rl_repo", "/root/.axon_site/_ro/trn_rl_repo"):
    if os.path.isdir(_p) and _p not in sys.path:
        sys.path.insert(0, _p)

import numpy as np
from contextlib import ExitStack

import concourse.bass as bass
import concourse.bass_isa as bass_isa
import concourse.mybir as mybir
import concourse.tile as tile
from concourse.bass_utils import run_bass_kernel_spmd

dt = mybir.dt
AF = mybir.ActivationFunctionType
ALU = mybir.AluOpType

MAGIC = float(1.5 * 2**23)  # fp32 round-to-nearest-even magic constant
D = 1024
F = 4096
L = 4
NCORES = 8
TOK = 1024  # tokens per core (one batch of S=1024)

NDK = D // 128   # 8 contraction tiles for up-proj
NFT = F // 128   # 32 contraction tiles for down-proj
NFC = F // 512   # 8 free-dim chunks for up-proj
NCH = F // 1024  # 4 column-chunks in the repacked wg/wu layout
NTT = TOK // 128  # 8 token tiles


def _split_excess_waits(nc, max_waits: int = 1) -> int:
    """walrus in this container rejects >1 sync-wait per instruction; split
    extras into standalone event-semaphore waits on the same engine (same-
    engine program order makes this semantically identical)."""
    n = 0
    for func in nc.m.functions:
        for block in func.blocks:
            changed = False
            out = []
            for inst in block.instructions:
                si = getattr(inst, "sync_info", None)
                if si is not None and si.on_wait and len(si.on_wait) > max_waits:
                    waits = list(si.on_wait)
                    for j, w in enumerate(waits[max_waits:]):
                        out.append(
                            mybir.InstEventSemaphore(
                                name=f"{inst.name}-xw{j}",
                                engine=inst.engine,
                                ins=[],
                                outs=[],
                                sync_info=mybir.SyncInfo(on_wait=[w], on_update=[]),
                            )
                        )
                        n += 1
                    inst.sync_info = mybir.SyncInfo(
                        on_wait=waits[:max_waits], on_update=list(si.on_update)
                    )
                    changed = True
                out.append(inst)
            if changed:
                block.instructions = out
    return n


def build(is_ones: bool = True, n_cores: int = NCORES, n_tok_tiles: int = NTT,
          n_layers: int = L) -> bass.Bass:
    nc = bass.Bass(num_devices=n_cores)
    tok = n_tok_tiles * 128

    x_ext = nc.declare_dram_parameter("x", [tok, D], dt.float32, isOutput=False)
    rs_ext = nc.declare_dram_parameter("rs", [n_layers, D], dt.float32, isOutput=False)
    # wg/wu repacked host-side to [L, F//1024, NDK, 128, 1024] so every
    # [128, 1024] weight tile is one contiguous 512 KB DMA; wd's row-slabs
    # are naturally contiguous.
    wg_ext = nc.declare_dram_parameter("wg", [n_layers, NCH, NDK, 128, 1024], dt.float32, isOutput=False)
    wu_ext = nc.declare_dram_parameter("wu", [n_layers, NCH, NDK, 128, 1024], dt.float32, isOutput=False)
    wd_ext = nc.declare_dram_parameter("wd", [n_layers, F, D], dt.float32, isOutput=False)
    # per-core row-slices of each matrix for the cooperative |W|max,
    # as [L, nun, 128, 1024] contiguous units
    nun = (D // n_cores) * F // (128 * 1024)
    slg_ext = nc.declare_dram_parameter("slg", [n_layers, nun, 128, 1024], dt.float32, isOutput=False)
    slu_ext = nc.declare_dram_parameter("slu", [n_layers, nun, 128, 1024], dt.float32, isOutput=False)
    sld_ext = nc.declare_dram_parameter("sld", [n_layers, nun, 128, 1024], dt.float32, isOutput=False)
    out_ext = nc.declare_dram_parameter("out", [tok, D], dt.float32, isOutput=True)

    mx_locs = [nc.dram_tensor(f"mx{l}_loc", [1, 16], dt.float32) for l in range(n_layers)]
    mx_globs = [nc.dram_tensor(f"mx{l}_glob", [1, 16], dt.float32) for l in range(n_layers)]
    # AllGather outputs for cooperative ternarize of layers >= 1:
    # [core, 128, 12288] fp8 where cols 0:4096 = wg (ch-major), 4096:8192 = wu,
    # 8192:12288 = wd (4 ft units)
    gall = [None] + [
        nc.dram_tensor(f"gall{l}", [n_cores, 128, 12288], dt.float8e4, addr_space="Shared")
        for l in range(1, n_layers)
    ]

    with tile.TileContext(nc) as tc, ExitStack() as ctx:
        P = ctx.enter_context
        wch = P(tc.tile_pool(name="wch", bufs=2 * NCH))   # per-ch wg/wu fp8 tiles
        wdp = P(tc.tile_pool(name="wdp", bufs=1))         # wd fp8 tile
        wstream = P(tc.tile_pool(name="wstream", bufs=2))
        wi32 = P(tc.tile_pool(name="wi32", bufs=2))
        xpool = P(tc.tile_pool(name="x1", bufs=5))
        t1pool = P(tc.tile_pool(name="t1", bufs=1))
        s4k = P(tc.tile_pool(name="s4k", bufs=2))         # xnew scratch
        qpool = P(tc.tile_pool(name="q", bufs=2))
        qtpool = P(tc.tile_pool(name="qt", bufs=2))
        silupool = P(tc.tile_pool(name="silu", bufs=2))   # [128,512] silu chunks
        scrpool = P(tc.tile_pool(name="scr", bufs=1))     # square scratch
        gpool = P(tc.tile_pool(name="g", bufs=1))
        gqpool = P(tc.tile_pool(name="gq", bufs=1))
        gqtpool = P(tc.tile_pool(name="gqt", bufs=2))
        w8s = P(tc.tile_pool(name="w8s", bufs=1))
        batch = P(tc.tile_pool(name="batch", bufs=2))     # [128, ntt] per-layer stats
        btmp = P(tc.tile_pool(name="btmp", bufs=1))       # stats chain temps
        sc = P(tc.tile_pool(name="sc", bufs=4))           # [128, small] scalars
        xdrpool = P(tc.tile_pool(name="xdr", bufs=1))
        const = P(tc.tile_pool(name="const", bufs=1))
        bcpool = P(tc.tile_pool(name="bc", bufs=2))       # rms_scale broadcast (general path)
        dram = P(tc.tile_pool(name="dram", bufs=2, space="DRAM"))
        ps_up = P(tc.tile_pool(name="psup", bufs=4, space="PSUM"))
        ps_dn = P(tc.tile_pool(name="psdn", bufs=4, space="PSUM"))

        # ---------- constants ----------
        mag = const.tile([128, 1], dt.float32, tag="mag")
        nc.gpsimd.memset(mag[:], MAGIC)
        ones1 = const.tile([1, 128], dt.float32, tag="ones1")
        nc.gpsimd.memset(ones1[:], 1.0)

        def pe_broadcast(dst, src_row, n):
            """broadcast src_row [1, n] to dst [128, n] via PE outer product."""
            for h in range(0, n, 512):
                w = min(512, n - h)
                bc_ps = ps_up.tile([128, 512], dt.float32, tag="ups")
                nc.tensor.matmul(bc_ps[:, 0:w], ones1[:], src_row[:, h:h + w],
                                 start=True, stop=True)
                nc.scalar.activation(dst[:, h:h + w], bc_ps[:, 0:w], AF.Copy)

        # ---------- cooperative per-matrix |W|max ----------
        # Layer 0's three matrices are reduced + allreduced first so its
        # ternarize can start while layers 1-3 slices still stream.
        wmax_cols = const.tile([128, 16], dt.float32, tag="wmaxc")
        nc.gpsimd.memset(wmax_cols[:], 0.0)

        def slab_max(l, ext, idx, eng):
            part = sc.tile([128, nun], dt.float32, tag="wmaxpart")
            for u in range(nun):
                wt = wstream.tile([128, 1024], dt.float32, tag="wstream")
                eng.dma_start(wt[:], ext[l, u])
                nc.vector.tensor_reduce(
                    part[:, u:u + 1], wt[:], axis=mybir.AxisListType.X,
                    op=ALU.max, apply_absolute_value=True,
                )
            nc.vector.tensor_reduce(
                wmax_cols[:, idx:idx + 1], part[:, 0:nun],
                axis=mybir.AxisListType.X, op=ALU.max,
                apply_absolute_value=False,
            )

        wscl = []
        for _l in range(n_layers):
            wsc_t = const.tile([128, 16], dt.float32, tag=f"wsc{_l}")
            wscl.append(wsc_t)
        grow = const.tile([1, 16], dt.float32, tag="grow")
        mrow = const.tile([1, 16], dt.float32, tag="mrow")
        nc.gpsimd.memset(mrow[:], 0.0)
        wsc_row = const.tile([1, 16], dt.float32, tag="wscrow")

        def finish_wsc(cols, loc, glob, wsc):
            """partition-reduce wmax cols, allreduce across cores, build
            wsc[:, cols] = 127/(m+1e-5) broadcast to all partitions."""
            a, b = cols
            nc.gpsimd.tensor_reduce(
                mrow[:, a:b], wmax_cols[:, a:b], axis=mybir.AxisListType.C, op=ALU.max
            )
            nc.scalar.dma_start(loc[:], mrow[:])
            nc.gpsimd.collective_compute(
                "AllReduce",
                ALU.max,
                replica_groups=[list(range(n_cores))],
                ins=[loc[:].opt()],
                outs=[glob[:].opt()],
            )
            nc.scalar.dma_start(grow[:, a:b], glob[:, a:b])
            nc.vector.tensor_scalar(wsc_row[:, a:b], grow[:, a:b], 1e-5, None, op0=ALU.add)
            nc.vector.reciprocal(wsc_row[:, a:b], wsc_row[:, a:b])
            nc.vector.tensor_scalar(wsc_row[:, a:b], wsc_row[:, a:b], 127.0, None, op0=ALU.mult)
            pe_broadcast(wsc, wsc_row, 16)

        for mi, ext in enumerate((slg_ext, slu_ext, sld_ext)):
            slab_max(0, ext, mi, nc.scalar)
        finish_wsc((0, 3), mx_locs[0], mx_globs[0], wscl[0])

        # ---------- ternarize ----------
        def tern_unit(src, dst_ap, idx, pass2_eng):
            """src: [128, 1024] fp32 AP; dst_ap: fp8 [128, 1024]-shaped."""
            wt = wstream.tile([128, 1024], dt.float32, tag="wstream")
            nc.sync.dma_start(wt[:], src)
            r32 = wi32.tile([128, 1024], dt.int32, tag="wi32")
            wsc = wscl[idx // 3]
            nc.vector.tensor_scalar(
                r32[:], wt[:], wsc[:, idx:idx + 1], None, op0=ALU.mult
            )
            if pass2_eng == "act":
                # sign(n) == clip(n, -1, 1) for integer n
                nc.scalar.activation(dst_ap, r32[:], AF.Sign)
            else:
                nc.gpsimd.tensor_scalar(dst_ap, r32[:], 1, -1, op0=ALU.min, op1=ALU.max)

        def tern_layer(l, dst_g, dst_u, dst_d, via_dram, split_pass2=False):
            """dst_g/dst_u: fn(ch, dk) -> fp8 [128,1024] AP; dst_d: fn(ft) -> AP."""
            cnt = [0]

            def unit(src, dst_ap, idx):
                eng = "act" if (split_pass2 and cnt[0] % 2 == 0) else "gp"
                cnt[0] += 1
                if via_dram:
                    stg = w8s.tile([128, 1024], dt.float8e4, tag="w8s")
                    tern_unit(src, stg[:], idx, eng)
                    nc.sync.dma_start(dst_ap, stg[:])
                else:
                    tern_unit(src, dst_ap, idx, eng)

            for ch in range(NCH):
                for dk in range(NDK):
                    unit(wg_ext[l, ch, dk], dst_g(ch, dk), 3 * l)
                    unit(wu_ext[l, ch, dk], dst_u(ch, dk), 3 * l + 1)
            for ft in range(NFT):
                unit(wd_ext[l, ft * 128:(ft + 1) * 128, :], dst_d(ft), 3 * l + 2)

        # ---------- cooperative weight prefetch (layers >= 1) ----------
        # Each core reduces + ternarizes only its 1/8 shard (the same
        # host-staged slices used for the |W|max pass), then one fp8
        # AllGather per layer shares the ternary weights; the gather hides
        # under the previous layer's compute.
        def coop_prefetch(lp):
            for mi, ext in enumerate((slg_ext, slu_ext, sld_ext)):
                slab_max(lp, ext, 3 * lp + mi, nc.sync)
            finish_wsc((3 * lp, 3 * lp + 3), mx_locs[lp], mx_globs[lp], wscl[lp])
            loc_all = dram.tile([128, 12288], dt.float8e4, tag="locall")
            for mi, ext in enumerate((slg_ext, slu_ext, sld_ext)):
                for u in range(nun):
                    stg = w8s.tile([128, 1024], dt.float8e4, tag="w8s")
                    tern_unit(ext[lp, u], stg[:], 3 * lp + mi, "gp")
                    nc.sync.dma_start(
                        loc_all[:, mi * 4096 + u * 1024:mi * 4096 + (u + 1) * 1024],
                        stg[:],
                    )
            nc.gpsimd.collective_compute(
                "AllGather",
                ALU.bypass,
                replica_groups=[list(range(n_cores))],
                ins=[loc_all[:].opt()],
                outs=[gall[lp][:].opt()],
            )

        # ---------- per-layer stats ----------
        # ssq/mx filled per token tile; the scalar chain runs on column
        # groups [0:4] and [4:8] as soon as those tiles' stats exist.
        def stats_chain(ssq_all, mx_all, c1_all, rs_all, a, b):
            k = b - a
            ms_t = btmp.tile([128, n_tok_tiles], dt.float32, tag="ms")
            rt_t = btmp.tile([128, n_tok_tiles], dt.float32, tag="rt")
            rstd_t = btmp.tile([128, n_tok_tiles], dt.float32, tag="rstd")
            nwt_t = btmp.tile([128, n_tok_tiles], dt.float32, tag="nwt")
            maxh_t = btmp.tile([128, n_tok_tiles], dt.float32, tag="maxh")
            sr_t = btmp.tile([128, n_tok_tiles], dt.float32, tag="sr")
            s_t = btmp.tile([128, n_tok_tiles], dt.float32, tag="s_")
            ms, rt, rstd = ms_t[:, 0:k], rt_t[:, 0:k], rstd_t[:, 0:k]
            nwt, maxh, sr, s_ = nwt_t[:, 0:k], maxh_t[:, 0:k], sr_t[:, 0:k], s_t[:, 0:k]
            nc.vector.tensor_scalar(ms, ssq_all[:, a:b], 1.0 / D, 1e-6, op0=ALU.mult, op1=ALU.add)
            nc.scalar.activation(rt, ms, AF.Sqrt)
            nc.vector.reciprocal(rstd, rt)
            # one Newton step: rstd *= 1.5 - 0.5*ms*rstd^2  (fixes the ~7e-6
            # Sqrt-LUT error that quantization tie-flips amplify layer by layer)
            nc.vector.tensor_tensor(nwt, rstd, rstd, op=ALU.mult)
            nc.vector.tensor_tensor(nwt, nwt, ms, op=ALU.mult)
            nc.vector.tensor_scalar(nwt, nwt, -0.5, 1.5, op0=ALU.mult, op1=ALU.add)
            nc.vector.tensor_tensor(rstd, rstd, nwt, op=ALU.mult)
            nc.vector.tensor_tensor(maxh, mx_all[:, a:b], rstd, op=ALU.mult)
            nc.vector.tensor_scalar(maxh, maxh, 1e-5, None, op0=ALU.add)
            nc.vector.reciprocal(sr, maxh)
            nc.vector.tensor_scalar(s_, sr, 127.0, 1e3, op0=ALU.mult, op1=ALU.min)
            nc.vector.tensor_scalar(s_, s_, 1e-3, None, op0=ALU.max)
            nc.vector.tensor_tensor(c1_all[:, a:b], s_, rstd, op=ALU.mult)
            nc.vector.reciprocal(rs_all[:, a:b], s_)

        def tile_stats(src_ap, ssq_all, mx_all, i, scale_bc):
            """ssq + scaled abs-max for token tile i of the NEXT layer's input."""
            scr = scrpool.tile([128, D], dt.float32, tag="scr")
            nc.scalar.activation(scr[:], src_ap, AF.Square, accum_out=ssq_all[:, i:i + 1])
            if is_ones:
                nc.vector.tensor_reduce(
                    mx_all[:, i:i + 1], src_ap, axis=mybir.AxisListType.X,
                    op=ALU.max, apply_absolute_value=True,
                )
            else:
                nc.vector.tensor_tensor(scr[:], src_ap, scale_bc[:], op=ALU.mult)
                nc.vector.tensor_reduce(
                    mx_all[:, i:i + 1], scr[:], axis=mybir.AxisListType.X,
                    op=ALU.max, apply_absolute_value=True,
                )

        # ---------- layers ----------
        xsrc = x_ext
        prev_stats = None  # (ssq, mx, c1, rs) for current layer, from fused pass

        for l in range(n_layers):
            wg_c, wu_c = [], []
            for _ch in range(NCH):
                wgc_t = wch.tile([128, NDK, 1024], dt.float8e4, tag="wch")
                wg_c.append(wgc_t)
            for _ch in range(NCH):
                wuc_t = wch.tile([128, NDK, 1024], dt.float8e4, tag="wch")
                wu_c.append(wuc_t)
            wd_t = wdp.tile([128, NFT, D], dt.float8e4, tag="wdp")
            if l == 0:
                tern_layer(
                    0,
                    lambda ch, dk: wg_c[ch][:, dk, :],
                    lambda ch, dk: wu_c[ch][:, dk, :],
                    lambda ft: wd_t[:, ft, :],
                    via_dram=False,
                    split_pass2=True,
                )
            else:
                ga = gall[l]
                # fill in consumption order: wg/wu chunk-columns, then wd
                for ch in range(NCH):
                    nc.sync.dma_start(
                        wg_c[ch][:],
                        ga[:, :, ch * 1024:(ch + 1) * 1024].transpose([1, 0, 2]))
                    nc.sync.dma_start(
                        wu_c[ch][:],
                        ga[:, :, 4096 + ch * 1024:4096 + (ch + 1) * 1024].transpose([1, 0, 2]))
                for c in range(n_cores):
                    nc.sync.dma_start(wd_t[:, 4 * c:4 * c + 4, :], ga[c, :, 8192:12288])

            scale_bc = None
            scale_bc_next = None
            if not is_ones:
                rs_row = const.tile([1, D], dt.float32, tag=f"rsrow{l}")
                nc.sync.dma_start(rs_row[:], rs_ext[l:l + 1, :])
                scale_bc = bcpool.tile([128, D], dt.float32, tag="bc")
                pe_broadcast(scale_bc, rs_row[:], D)
                if l + 1 < n_layers:
                    rs_row_n = const.tile([1, D], dt.float32, tag=f"rsrown{l}")
                    nc.sync.dma_start(rs_row_n[:], rs_ext[l + 1:l + 2, :])
                    scale_bc_next = bcpool.tile([128, D], dt.float32, tag="bcn")
                    pe_broadcast(scale_bc_next, rs_row_n[:], D)

            if l == 0:
                # phase A for layer 0 only: standalone stats pass
                ssq_all = batch.tile([128, n_tok_tiles], dt.float32, tag="ssq")
                mx_all = batch.tile([128, n_tok_tiles], dt.float32, tag="mx")
                c1_all = batch.tile([128, n_tok_tiles], dt.float32, tag="c1")
                rs_all = batch.tile([128, n_tok_tiles], dt.float32, tag="rs_all")
                for i in range(n_tok_tiles):
                    xa = xpool.tile([128, D], dt.float32, tag="x1")
                    nc.scalar.dma_start(xa[:], xsrc[i * 128:(i + 1) * 128, :])
                    tile_stats(xa[:], ssq_all, mx_all, i, scale_bc)
                    if i == 3:
                        stats_chain(ssq_all, mx_all, c1_all, rs_all, 0, 4)
                    elif i == n_tok_tiles - 1:
                        stats_chain(ssq_all, mx_all, c1_all, rs_all, 4, n_tok_tiles)
            else:
                ssq_all, mx_all, c1_all, rs_all = prev_stats

            if l + 1 < n_layers:
                ssq_n = batch.tile([128, n_tok_tiles], dt.float32, tag="ssq")
                mx_n = batch.tile([128, n_tok_tiles], dt.float32, tag="mx")
                c1_n = batch.tile([128, n_tok_tiles], dt.float32, tag="c1")
                rs_n = batch.tile([128, n_tok_tiles], dt.float32, tag="rs_all")
                prev_stats = (ssq_n, mx_n, c1_n, rs_n)

            xdst = out_ext if l == n_layers - 1 else dram.tile([tok, D], dt.float32, tag="xbuf")

            # per-tile state
            stq = [None] * n_tok_tiles   # (x1, qT) from emit_q
            std = [None] * n_tok_tiles   # (x1, gqT, rs2) from emit_mm

            def emit_q(i):
                """load x tile, quantize, transpose -- one tile ahead of the MMs."""
                x1 = xpool.tile([128, D], dt.float32, tag="x1")
                nc.scalar.dma_start(x1[:], xsrc[i * 128:(i + 1) * 128, :])
                t1 = t1pool.tile([128, D], dt.float32, tag="t1")
                # q = round(c1 * h') via magic add (ACT) / subtract (DVE), out bf16
                if is_ones:
                    nc.scalar.activation(t1[:], x1[:], AF.Identity,
                                         scale=c1_all[:, i:i + 1], bias=mag[:])
                else:
                    nc.vector.tensor_tensor(t1[:], x1[:], scale_bc[:], op=ALU.mult)
                    nc.scalar.activation(t1[:], t1[:], AF.Identity,
                                         scale=c1_all[:, i:i + 1], bias=mag[:])
                q = qpool.tile([128, D], dt.bfloat16, tag="q")
                nc.vector.tensor_scalar(q[:], t1[:], MAGIC, None, op0=ALU.subtract)
                qT = qtpool.tile([128, NDK, 128], dt.bfloat16, tag="qt")
                nc.scalar.dma_start_transpose(qT[:], q[:])
                stq[i] = (x1, qT)

            def emit_mm(i):
                """up-proj matmuls + GLU + act-quant + gqT for tile i."""
                x1, qT = stq[i]
                g = gpool.tile([128, F], dt.float32, tag="g")
                gm8 = sc.tile([128, NFC], dt.float32, tag="gm8")
                for f in range(NFC):
                    ch = f // 2
                    fo = (f % 2) * 512
                    u_ps = ps_up.tile([128, 512], dt.float32, tag="ups")
                    v_ps = ps_up.tile([128, 512], dt.float32, tag="ups")
                    for dk in range(NDK):
                        nc.tensor.matmul(
                            u_ps[:], qT[:, dk, :], wg_c[ch][:, dk, fo:fo + 512],
                            start=(dk == 0), stop=(dk == NDK - 1),
                        )
                        nc.tensor.matmul(
                            v_ps[:], qT[:, dk, :], wu_c[ch][:, dk, fo:fo + 512],
                            start=(dk == 0), stop=(dk == NDK - 1),
                        )
                    su = silupool.tile([128, 512], dt.float32, tag="silu")
                    nc.scalar.activation(su[:], u_ps[:], AF.Silu,
                                         scale=rs_all[:, i:i + 1])
                    nc.vector.tensor_tensor(
                        g[:, f * 512:(f + 1) * 512], su[:], v_ps[:], op=ALU.mult
                    )
                    nc.vector.tensor_reduce(
                        gm8[:, f:f + 1], g[:, f * 512:(f + 1) * 512],
                        axis=mybir.AxisListType.X, op=ALU.max, apply_absolute_value=True,
                    )
                # s2 = clip(127/(max|g|/s + 1e-5)); c2 = s2/s ; rs2 = 1/s2
                gmx = sc.tile([128, 1], dt.float32, tag="gmx")
                nc.vector.tensor_reduce(
                    gmx[:], gm8[:], axis=mybir.AxisListType.X, op=ALU.max,
                    apply_absolute_value=False,
                )
                nc.vector.tensor_tensor(gmx[:], gmx[:], rs_all[:, i:i + 1], op=ALU.mult)
                nc.vector.tensor_scalar(gmx[:], gmx[:], 1e-5, None, op0=ALU.add)
                s2r = sc.tile([128, 1], dt.float32, tag="s2r")
                nc.vector.reciprocal(s2r[:], gmx[:])
                s2 = sc.tile([128, 1], dt.float32, tag="s2")
                nc.vector.tensor_scalar(s2[:], s2r[:], 127.0, 1e3, op0=ALU.mult, op1=ALU.min)
                nc.vector.tensor_scalar(s2[:], s2[:], 1e-3, None, op0=ALU.max)
                c2 = sc.tile([128, 1], dt.float32, tag="c2")
                nc.vector.tensor_tensor(c2[:], s2[:], rs_all[:, i:i + 1], op=ALU.mult)
                rs2 = sc.tile([128, 1], dt.float32, tag="rs2")
                nc.vector.reciprocal(rs2[:], s2[:])
                # gq = round(c2*g) in two halves (ACT magic pass in place, DVE
                # subtract to bf16, transpose) so the serial tail is half as long
                gq = gqpool.tile([128, F], dt.bfloat16, tag="gq")
                gqT = gqtpool.tile([128, NFT, 128], dt.bfloat16, tag="gqt")
                for h in range(2):
                    hs = h * (F // 2)
                    he = hs + F // 2
                    nc.scalar.activation(g[:, hs:he], g[:, hs:he], AF.Identity,
                                         scale=c2[:], bias=mag[:])
                    nc.vector.tensor_scalar(gq[:, hs:he], g[:, hs:he], MAGIC, None,
                                            op0=ALU.subtract)
                    nc.scalar.dma_start_transpose(
                        gqT[:, h * (NFT // 2):(h + 1) * (NFT // 2), :], gq[:, hs:he])
                std[i] = (x1, gqT, rs2)
                stq[i] = None

            def emit_down(i):
                x1, gqT, rs2 = std[i]
                xd_ps0 = ps_dn.tile([128, 512], dt.float32, tag="dps")
                xd_ps1 = ps_dn.tile([128, 512], dt.float32, tag="dps")
                for ft in range(NFT):
                    nc.tensor.matmul(
                        xd_ps0[:], gqT[:, ft, :], wd_t[:, ft, 0:512],
                        start=(ft == 0), stop=(ft == NFT - 1),
                    )
                    nc.tensor.matmul(
                        xd_ps1[:], gqT[:, ft, :], wd_t[:, ft, 512:1024],
                        start=(ft == 0), stop=(ft == NFT - 1),
                    )
                xnew = s4k.tile([128, D], dt.float32, tag="s4k")
                for dc, xd_ps in ((0, xd_ps0), (1, xd_ps1)):
                    xdr = xdrpool.tile([128, 512], dt.float32, tag="xdr")
                    nc.scalar.activation(xdr[:], xd_ps[:], AF.Copy, scale=rs2[:])
                    nc.vector.tensor_tensor(
                        xnew[:, dc * 512:(dc + 1) * 512],
                        x1[:, dc * 512:(dc + 1) * 512], xdr[:], op=ALU.add,
                    )
                if l + 1 < n_layers:
                    tile_stats(xnew[:], ssq_n, mx_n, i, scale_bc_next)
                    if i == 3:
                        stats_chain(ssq_n, mx_n, c1_n, rs_n, 0, 4)
                    elif i == n_tok_tiles - 1:
                        stats_chain(ssq_n, mx_n, c1_n, rs_n, 4, n_tok_tiles)
                nc.scalar.dma_start(xdst[i * 128:(i + 1) * 128, :], xnew[:])
                std[i] = None

            # software pipeline with one-tile lookahead on q/qT:
            # q0 q1 | mm0 q2 | mm1 q3 dn0 | mm2 q4 dn1 | ... | mm7 dn6 | dn7
            emit_q(0)
            emit_q(1)
            for i in range(n_tok_tiles):
                emit_mm(i)
                if i + 2 < n_tok_tiles:
                    emit_q(i + 2)
                if i >= 1:
                    emit_down(i - 1)
                if i == 3 and l + 1 < n_layers:
                    coop_prefetch(l + 1)
            emit_down(n_tok_tiles - 1)

            xsrc = xdst

    _split_excess_waits(nc)
    return nc


_nc_cache = {}


def _get_nc(key):
    if key not in _nc_cache:
        _nc_cache[key] = build(*key)
    return _nc_cache[key]


def _repack(w, n_layers):
    # [L, D, F] -> [L, F//1024, D//128, 128, 1024] so each (ch, dk) tile is
    # contiguous; dk == core index for the per-core max slices.
    return np.ascontiguousarray(
        w.reshape(n_layers, D // 128, 128, NCH, 1024).transpose(0, 3, 1, 2, 4)
    )


def _make_in_maps(x, rs, wg, wu, wd, n_cores=NCORES):
    n_layers = rs.shape[0]
    wg_r = _repack(wg, n_layers)
    wu_r = _repack(wu, n_layers)
    nsl = NDK // n_cores  # dk-slabs per core for slg/slu
    dsl = F // n_cores
    in_maps = []
    for c in range(n_cores):
        slg = wg_r[:, :, c * nsl:(c + 1) * nsl].reshape(n_layers, -1, 128, 1024)
        slu = wu_r[:, :, c * nsl:(c + 1) * nsl].reshape(n_layers, -1, 128, 1024)
        sld = wd[:, c * dsl:(c + 1) * dsl, :].reshape(n_layers, -1, 128, 1024)
        in_maps.append({
            "x": x[c],
            "rs": rs,
            "wg": wg_r,
            "wu": wu_r,
            "wd": wd,
            "slg": np.ascontiguousarray(slg),
            "slu": np.ascontiguousarray(slu),
            "sld": np.ascontiguousarray(sld),
        })
    return in_maps


def kernel(x, rms_scale, W_g, W_u, W_d):
    """Full-input entry point: shard over batch, run 8-core SPMD, gather."""
    x = np.ascontiguousarray(np.asarray(x, dtype=np.float32))
    rs = np.ascontiguousarray(np.asarray(rms_scale, dtype=np.float32))
    wg = np.ascontiguousarray(np.asarray(W_g, dtype=np.float32))
    wu = np.ascontiguousarray(np.asarray(W_u, dtype=np.float32))
    wd = np.ascontiguousarray(np.asarray(W_d, dtype=np.float32))
    B, S, Dx = x.shape
    assert (B, S, Dx) == (NCORES, TOK, D), (B, S, Dx)
    is_ones = bool(np.all(rs == 1.0))
    nc = _get_nc((is_ones,))
    in_maps = _make_in_maps(x, rs, wg, wu, wd)
    res = run_bass_kernel_spmd(nc, in_maps, list(range(NCORES)))
    return np.stack([res.results[c]["out"] for c in range(NCORES)], axis=0)


# revision 18
# speedup vs baseline: 1.2101x; 1.0146x over previous
"""Trainium2 Bass kernel for nn_CascadeTransformerMM (4-layer ternary-GLU cascade).

Math (per layer, per token row):
  h   = rms_scale * x * rsqrt(mean(x^2) + 1e-6)
  s   = clip(127/(max|h| + 1e-5), 1e-3, 1e3);  q = round(s*h)      (ints in [-127,127])
  Wt  = clip(round(W * 127/(max|W| + 1e-5)), -1, 1)                 (ternary {-1,0,1})
  u   = (q @ Wg_t)/s ; v = (q @ Wu_t)/s ; g = silu(u)*v
  s2  = clip(127/(max|g| + 1e-5), 1e-3, 1e3); gq = round(s2*g)
  x  += (gq @ Wd_t)/s2

Distribution: pure data-parallel over the batch dim (8 batches -> 8 cores),
weights replicated per core. Per-matrix |W|max is computed cooperatively:
each core reduces a 1/8 row-slice, then a tiny AllReduce(max) shares the 12
scalars (layer 0's three matrices allreduced first so its ternarize can
start early). All matmuls run on the PE array with bf16 activations
(integers <= 127, exact) x fp8 ternary weights (exact), fp32 PSUM
accumulation -> the heavy compute is bit-exact integer arithmetic.

Schedule: per-token-tile software pipeline with one-tile lookahead --
q/qT production for tile i+2 and down-proj for tile i-1 are emitted
around the up-proj matmul block of tile i, so the serial per-tile tail
(s2 chain -> gq round -> gqT transpose) hides under the next tile's
matmuls and the PE stream stays dense (keeps the HAM clock-gate warm).
Weights live in per-column-chunk SBUF tiles so layer-0 ternarize and
layer-boundary refills unblock the matmul stream progressively.
Ternarize runs pass 1 (scale+round-to-int32) on DVE and pass 2 (clip to
{-1,0,1} as fp8) on the otherwise-idle GPSIMD (alternating with ACT
Sign for layer 0, where latency matters)."""

import os
import sys

for _p in ("/opt/trn_rl_repo", "/root/.axon_site/_ro/trn_rl_repo"):
    if os.path.isdir(_p) and _p not in sys.path:
        sys.path.insert(0, _p)

import numpy as np
from contextlib import ExitStack

import concourse.bass as bass
import concourse.bass_isa as bass_isa
import concourse.mybir as mybir
import concourse.tile as tile
from concourse.bass_utils import run_bass_kernel_spmd

dt = mybir.dt
AF = mybir.ActivationFunctionType
ALU = mybir.AluOpType

MAGIC = float(1.5 * 2**23)  # fp32 round-to-nearest-even magic constant
D = 1024
F = 4096
L = 4
NCORES = 8
TOK = 1024  # tokens per core (one batch of S=1024)

NDK = D // 128   # 8 contraction tiles for up-proj
NFT = F // 128   # 32 contraction tiles for down-proj
NFC = F // 512   # 8 free-dim chunks for up-proj
NCH = F // 1024  # 4 column-chunks in the repacked wg/wu layout
NTT = TOK // 128  # 8 token tiles


def _split_excess_waits(nc, max_waits: int = 1) -> int:
    """walrus in this container rejects >1 sync-wait per instruction; split
    extras into standalone event-semaphore waits on the same engine (same-
    engine program order makes this semantically identical)."""
    n = 0
    for func in nc.m.functions:
        for block in func.blocks:
            changed = False
            out = []
            for inst in block.instructions:
                si = getattr(inst, "sync_info", None)
                if si is not None and si.on_wait and len(si.on_wait) > max_waits:
                    waits = list(si.on_wait)
                    for j, w in enumerate(waits[max_waits:]):
                        out.append(
                            mybir.InstEventSemaphore(
                                name=f"{inst.name}-xw{j}",
                                engine=inst.engine,
                                ins=[],
                                outs=[],
                                sync_info=mybir.SyncInfo(on_wait=[w], on_update=[]),
                            )
                        )
                        n += 1
                    inst.sync_info = mybir.SyncInfo(
                        on_wait=waits[:max_waits], on_update=list(si.on_update)
                    )
                    changed = True
                out.append(inst)
            if changed:
                block.instructions = out
    return n


def build(is_ones: bool = True, n_cores: int = NCORES, n_tok_tiles: int = NTT,
          n_layers: int = L) -> bass.Bass:
    nc = bass.Bass(num_devices=n_cores)
    tok = n_tok_tiles * 128

    x_ext = nc.declare_dram_parameter("x", [tok, D], dt.float32, isOutput=False)
    rs_ext = nc.declare_dram_parameter("rs", [n_layers, D], dt.float32, isOutput=False)
    # wg/wu repacked host-side to [L, F//1024, NDK, 128, 1024] so every
    # [128, 1024] weight tile is one contiguous 512 KB DMA; wd's row-slabs
    # are naturally contiguous.
    wg_ext = nc.declare_dram_parameter("wg", [n_layers, NCH, NDK, 128, 1024], dt.float32, isOutput=False)
    wu_ext = nc.declare_dram_parameter("wu", [n_layers, NCH, NDK, 128, 1024], dt.float32, isOutput=False)
    wd_ext = nc.declare_dram_parameter("wd", [n_layers, F, D], dt.float32, isOutput=False)
    # per-core row-slices of each matrix for the cooperative |W|max,
    # as [L, nun, 128, 1024] contiguous units
    nun = (D // n_cores) * F // (128 * 1024)
    slg_ext = nc.declare_dram_parameter("slg", [n_layers, nun, 128, 1024], dt.float32, isOutput=False)
    slu_ext = nc.declare_dram_parameter("slu", [n_layers, nun, 128, 1024], dt.float32, isOutput=False)
    sld_ext = nc.declare_dram_parameter("sld", [n_layers, nun, 128, 1024], dt.float32, isOutput=False)
    out_ext = nc.declare_dram_parameter("out", [tok, D], dt.float32, isOutput=True)

    mx_locs = [nc.dram_tensor(f"mx{l}_loc", [1, 16], dt.float32) for l in range(n_layers)]
    mx_globs = [nc.dram_tensor(f"mx{l}_glob", [1, 16], dt.float32) for l in range(n_layers)]
    # AllGather outputs for cooperative ternarize of layers >= 1:
    # [core, 128, 12288] fp8 where cols 0:4096 = wg (ch-major), 4096:8192 = wu,
    # 8192:12288 = wd (4 ft units)
    gall = [
        nc.dram_tensor(f"gall{l}", [n_cores, 128, 12288], dt.float8e4, addr_space="Shared")
        for l in range(n_layers)
    ]

    with tile.TileContext(nc) as tc, ExitStack() as ctx:
        P = ctx.enter_context
        wch = P(tc.tile_pool(name="wch", bufs=2 * NCH))   # per-ch wg/wu fp8 tiles
        wdp = P(tc.tile_pool(name="wdp", bufs=1))         # wd fp8 tile
        wstream = P(tc.tile_pool(name="wstream", bufs=2))
        wi32 = P(tc.tile_pool(name="wi32", bufs=2))
        xpool = P(tc.tile_pool(name="x1", bufs=5))
        t1pool = P(tc.tile_pool(name="t1", bufs=1))
        s4k = P(tc.tile_pool(name="s4k", bufs=2))         # xnew scratch
        qpool = P(tc.tile_pool(name="q", bufs=2))
        qtpool = P(tc.tile_pool(name="qt", bufs=2))
        silupool = P(tc.tile_pool(name="silu", bufs=2))   # [128,512] silu chunks
        scrpool = P(tc.tile_pool(name="scr", bufs=1))     # square scratch
        gpool = P(tc.tile_pool(name="g", bufs=1))
        gqpool = P(tc.tile_pool(name="gq", bufs=1))
        gqtpool = P(tc.tile_pool(name="gqt", bufs=2))
        w8s = P(tc.tile_pool(name="w8s", bufs=1))
        batch = P(tc.tile_pool(name="batch", bufs=2))     # [128, ntt] per-layer stats
        btmp = P(tc.tile_pool(name="btmp", bufs=1))       # stats chain temps
        sc = P(tc.tile_pool(name="sc", bufs=4))           # [128, small] scalars
        xdrpool = P(tc.tile_pool(name="xdr", bufs=1))
        const = P(tc.tile_pool(name="const", bufs=1))
        bcpool = P(tc.tile_pool(name="bc", bufs=2))       # rms_scale broadcast (general path)
        dram = P(tc.tile_pool(name="dram", bufs=2, space="DRAM"))
        ps_up = P(tc.tile_pool(name="psup", bufs=4, space="PSUM"))
        ps_dn = P(tc.tile_pool(name="psdn", bufs=4, space="PSUM"))

        # ---------- constants ----------
        mag = const.tile([128, 1], dt.float32, tag="mag")
        nc.gpsimd.memset(mag[:], MAGIC)
        ones1 = const.tile([1, 128], dt.float32, tag="ones1")
        nc.gpsimd.memset(ones1[:], 1.0)

        def pe_broadcast(dst, src_row, n):
            """broadcast src_row [1, n] to dst [128, n] via PE outer product."""
            for h in range(0, n, 512):
                w = min(512, n - h)
                bc_ps = ps_up.tile([128, 512], dt.float32, tag="ups")
                nc.tensor.matmul(bc_ps[:, 0:w], ones1[:], src_row[:, h:h + w],
                                 start=True, stop=True)
                nc.scalar.activation(dst[:, h:h + w], bc_ps[:, 0:w], AF.Copy)

        # ---------- cooperative per-matrix |W|max ----------
        # Layer 0's three matrices are reduced + allreduced first so its
        # ternarize can start while layers 1-3 slices still stream.
        wmax_cols = const.tile([128, 16], dt.float32, tag="wmaxc")
        nc.gpsimd.memset(wmax_cols[:], 0.0)

        def slab_max(l, ext, idx, eng):
            part = sc.tile([128, nun], dt.float32, tag="wmaxpart")
            for u in range(nun):
                wt = wstream.tile([128, 1024], dt.float32, tag="wstream")
                eng.dma_start(wt[:], ext[l, u])
                nc.vector.tensor_reduce(
                    part[:, u:u + 1], wt[:], axis=mybir.AxisListType.X,
                    op=ALU.max, apply_absolute_value=True,
                )
            nc.vector.tensor_reduce(
                wmax_cols[:, idx:idx + 1], part[:, 0:nun],
                axis=mybir.AxisListType.X, op=ALU.max,
                apply_absolute_value=False,
            )

        wscl = []
        for _l in range(n_layers):
            wsc_t = const.tile([128, 16], dt.float32, tag=f"wsc{_l}")
            wscl.append(wsc_t)
        grow = const.tile([1, 16], dt.float32, tag="grow")
        mrow = const.tile([1, 16], dt.float32, tag="mrow")
        nc.gpsimd.memset(mrow[:], 0.0)
        wsc_row = const.tile([1, 16], dt.float32, tag="wscrow")

        def finish_wsc(cols, loc, glob, wsc):
            """partition-reduce wmax cols, allreduce across cores, build
            wsc[:, cols] = 127/(m+1e-5) broadcast to all partitions."""
            a, b = cols
            nc.gpsimd.tensor_reduce(
                mrow[:, a:b], wmax_cols[:, a:b], axis=mybir.AxisListType.C, op=ALU.max
            )
            nc.scalar.dma_start(loc[:], mrow[:])
            nc.gpsimd.collective_compute(
                "AllReduce",
                ALU.max,
                replica_groups=[list(range(n_cores))],
                ins=[loc[:].opt()],
                outs=[glob[:].opt()],
            )
            nc.scalar.dma_start(grow[:, a:b], glob[:, a:b])
            nc.vector.tensor_scalar(wsc_row[:, a:b], grow[:, a:b], 1e-5, None, op0=ALU.add)
            nc.vector.reciprocal(wsc_row[:, a:b], wsc_row[:, a:b])
            nc.vector.tensor_scalar(wsc_row[:, a:b], wsc_row[:, a:b], 127.0, None, op0=ALU.mult)
            pe_broadcast(wsc, wsc_row, 16)

        # ---------- ternarize ----------
        def tern_unit(src, dst_ap, idx, pass2_eng="gp"):
            """src: [128, 1024] fp32 AP; dst_ap: fp8 [128, 1024]-shaped."""
            wt = wstream.tile([128, 1024], dt.float32, tag="wstream")
            nc.sync.dma_start(wt[:], src)
            r32 = wi32.tile([128, 1024], dt.int32, tag="wi32")
            wsc = wscl[idx // 3]
            nc.vector.tensor_scalar(
                r32[:], wt[:], wsc[:, idx:idx + 1], None, op0=ALU.mult
            )
            if pass2_eng == "act":
                # sign(n) == clip(n, -1, 1) for integer n
                nc.scalar.activation(dst_ap, r32[:], AF.Sign)
            else:
                nc.gpsimd.tensor_scalar(dst_ap, r32[:], 1, -1, op0=ALU.min, op1=ALU.max)

        # ---------- cooperative weight prefetch (layers >= 1) ----------
        # Each core reduces + ternarizes only its 1/8 shard (the same
        # host-staged slices used for the |W|max pass), then one fp8
        # AllGather per layer shares the ternary weights; the gather hides
        # under the previous layer's compute.
        def coop_prefetch(lp):
            for mi, ext in enumerate((slg_ext, slu_ext, sld_ext)):
                slab_max(lp, ext, 3 * lp + mi, nc.sync)
            finish_wsc((3 * lp, 3 * lp + 3), mx_locs[lp], mx_globs[lp], wscl[lp])
            loc_all = dram.tile([128, 12288], dt.float8e4, tag="locall")
            for mi, ext in enumerate((slg_ext, slu_ext, sld_ext)):
                for u in range(nun):
                    stg = w8s.tile([128, 1024], dt.float8e4, tag="w8s")
                    tern_unit(ext[lp, u], stg[:], 3 * lp + mi,
                              "act" if (mi * nun + u) % 2 == 0 else "gp")
                    nc.sync.dma_start(
                        loc_all[:, mi * 4096 + u * 1024:mi * 4096 + (u + 1) * 1024],
                        stg[:],
                    )
            nc.gpsimd.collective_compute(
                "AllGather",
                ALU.bypass,
                replica_groups=[list(range(n_cores))],
                ins=[loc_all[:].opt()],
                outs=[gall[lp][:].opt()],
            )

        # ---------- per-layer stats ----------
        # ssq/mx filled per token tile; the scalar chain runs on column
        # groups [0:4] and [4:8] as soon as those tiles' stats exist.
        def stats_chain(ssq_all, mx_all, c1_all, rs_all, a, b):
            k = b - a
            ms_t = btmp.tile([128, n_tok_tiles], dt.float32, tag="ms")
            rt_t = btmp.tile([128, n_tok_tiles], dt.float32, tag="rt")
            rstd_t = btmp.tile([128, n_tok_tiles], dt.float32, tag="rstd")
            nwt_t = btmp.tile([128, n_tok_tiles], dt.float32, tag="nwt")
            maxh_t = btmp.tile([128, n_tok_tiles], dt.float32, tag="maxh")
            sr_t = btmp.tile([128, n_tok_tiles], dt.float32, tag="sr")
            s_t = btmp.tile([128, n_tok_tiles], dt.float32, tag="s_")
            ms, rt, rstd = ms_t[:, 0:k], rt_t[:, 0:k], rstd_t[:, 0:k]
            nwt, maxh, sr, s_ = nwt_t[:, 0:k], maxh_t[:, 0:k], sr_t[:, 0:k], s_t[:, 0:k]
            nc.vector.tensor_scalar(ms, ssq_all[:, a:b], 1.0 / D, 1e-6, op0=ALU.mult, op1=ALU.add)
            nc.scalar.activation(rt, ms, AF.Sqrt)
            nc.vector.reciprocal(rstd, rt)
            # one Newton step: rstd *= 1.5 - 0.5*ms*rstd^2  (fixes the ~7e-6
            # Sqrt-LUT error that quantization tie-flips amplify layer by layer)
            nc.vector.tensor_tensor(nwt, rstd, rstd, op=ALU.mult)
            nc.vector.tensor_tensor(nwt, nwt, ms, op=ALU.mult)
            nc.vector.tensor_scalar(nwt, nwt, -0.5, 1.5, op0=ALU.mult, op1=ALU.add)
            nc.vector.tensor_tensor(rstd, rstd, nwt, op=ALU.mult)
            nc.vector.tensor_tensor(maxh, mx_all[:, a:b], rstd, op=ALU.mult)
            nc.vector.tensor_scalar(maxh, maxh, 1e-5, None, op0=ALU.add)
            nc.vector.reciprocal(sr, maxh)
            nc.vector.tensor_scalar(s_, sr, 127.0, 1e3, op0=ALU.mult, op1=ALU.min)
            nc.vector.tensor_scalar(s_, s_, 1e-3, None, op0=ALU.max)
            nc.vector.tensor_tensor(c1_all[:, a:b], s_, rstd, op=ALU.mult)
            nc.vector.reciprocal(rs_all[:, a:b], s_)

        def tile_stats(src_ap, ssq_all, mx_all, i, scale_bc):
            """ssq + scaled abs-max for token tile i of the NEXT layer's input."""
            scr = scrpool.tile([128, D], dt.float32, tag="scr")
            nc.scalar.activation(scr[:], src_ap, AF.Square, accum_out=ssq_all[:, i:i + 1])
            if is_ones:
                nc.vector.tensor_reduce(
                    mx_all[:, i:i + 1], src_ap, axis=mybir.AxisListType.X,
                    op=ALU.max, apply_absolute_value=True,
                )
            else:
                nc.vector.tensor_tensor(scr[:], src_ap, scale_bc[:], op=ALU.mult)
                nc.vector.tensor_reduce(
                    mx_all[:, i:i + 1], scr[:], axis=mybir.AxisListType.X,
                    op=ALU.max, apply_absolute_value=True,
                )

        # layer 0's weights: cooperative ternarize + gather up front
        coop_prefetch(0)

        # ---------- layers ----------
        xsrc = x_ext
        prev_stats = None  # (ssq, mx, c1, rs) for current layer, from fused pass

        for l in range(n_layers):
            wg_c, wu_c = [], []
            for _ch in range(NCH):
                wgc_t = wch.tile([128, NDK, 1024], dt.float8e4, tag="wch")
                wg_c.append(wgc_t)
            for _ch in range(NCH):
                wuc_t = wch.tile([128, NDK, 1024], dt.float8e4, tag="wch")
                wu_c.append(wuc_t)
            wd_t = wdp.tile([128, NFT, D], dt.float8e4, tag="wdp")
            ga = gall[l]
            # fill in consumption order: wg/wu chunk-columns, then wd
            for ch in range(NCH):
                nc.sync.dma_start(
                    wg_c[ch][:],
                    ga[:, :, ch * 1024:(ch + 1) * 1024].transpose([1, 0, 2]))
                nc.sync.dma_start(
                    wu_c[ch][:],
                    ga[:, :, 4096 + ch * 1024:4096 + (ch + 1) * 1024].transpose([1, 0, 2]))
            for c in range(n_cores):
                nc.sync.dma_start(wd_t[:, 4 * c:4 * c + 4, :], ga[c, :, 8192:12288])

            scale_bc = None
            scale_bc_next = None
            if not is_ones:
                rs_row = const.tile([1, D], dt.float32, tag=f"rsrow{l}")
                nc.sync.dma_start(rs_row[:], rs_ext[l:l + 1, :])
                scale_bc = bcpool.tile([128, D], dt.float32, tag="bc")
                pe_broadcast(scale_bc, rs_row[:], D)
                if l + 1 < n_layers:
                    rs_row_n = const.tile([1, D], dt.float32, tag=f"rsrown{l}")
                    nc.sync.dma_start(rs_row_n[:], rs_ext[l + 1:l + 2, :])
                    scale_bc_next = bcpool.tile([128, D], dt.float32, tag="bcn")
                    pe_broadcast(scale_bc_next, rs_row_n[:], D)

            if l == 0:
                # phase A for layer 0 only: standalone stats pass
                ssq_all = batch.tile([128, n_tok_tiles], dt.float32, tag="ssq")
                mx_all = batch.tile([128, n_tok_tiles], dt.float32, tag="mx")
                c1_all = batch.tile([128, n_tok_tiles], dt.float32, tag="c1")
                rs_all = batch.tile([128, n_tok_tiles], dt.float32, tag="rs_all")
                for i in range(n_tok_tiles):
                    xa = xpool.tile([128, D], dt.float32, tag="x1")
                    nc.scalar.dma_start(xa[:], xsrc[i * 128:(i + 1) * 128, :])
                    tile_stats(xa[:], ssq_all, mx_all, i, scale_bc)
                    if i == 3:
                        stats_chain(ssq_all, mx_all, c1_all, rs_all, 0, 4)
                    elif i == n_tok_tiles - 1:
                        stats_chain(ssq_all, mx_all, c1_all, rs_all, 4, n_tok_tiles)
            else:
                ssq_all, mx_all, c1_all, rs_all = prev_stats

            if l + 1 < n_layers:
                ssq_n = batch.tile([128, n_tok_tiles], dt.float32, tag="ssq")
                mx_n = batch.tile([128, n_tok_tiles], dt.float32, tag="mx")
                c1_n = batch.tile([128, n_tok_tiles], dt.float32, tag="c1")
                rs_n = batch.tile([128, n_tok_tiles], dt.float32, tag="rs_all")
                prev_stats = (ssq_n, mx_n, c1_n, rs_n)

            xdst = out_ext if l == n_layers - 1 else dram.tile([tok, D], dt.float32, tag="xbuf")

            # per-tile state
            stq = [None] * n_tok_tiles   # (x1, qT) from emit_q
            std = [None] * n_tok_tiles   # (x1, gqT, rs2) from emit_mm

            def emit_q(i):
                """load x tile, quantize, transpose -- one tile ahead of the MMs."""
                x1 = xpool.tile([128, D], dt.float32, tag="x1")
                nc.scalar.dma_start(x1[:], xsrc[i * 128:(i + 1) * 128, :])
                t1 = t1pool.tile([128, D], dt.float32, tag="t1")
                # q = round(c1 * h') via magic add (ACT) / subtract (DVE), out bf16
                if is_ones:
                    nc.scalar.activation(t1[:], x1[:], AF.Identity,
                                         scale=c1_all[:, i:i + 1], bias=mag[:])
                else:
                    nc.vector.tensor_tensor(t1[:], x1[:], scale_bc[:], op=ALU.mult)
                    nc.scalar.activation(t1[:], t1[:], AF.Identity,
                                         scale=c1_all[:, i:i + 1], bias=mag[:])
                q = qpool.tile([128, D], dt.bfloat16, tag="q")
                nc.vector.tensor_scalar(q[:], t1[:], MAGIC, None, op0=ALU.subtract)
                qT = qtpool.tile([128, NDK, 128], dt.bfloat16, tag="qt")
                nc.scalar.dma_start_transpose(qT[:], q[:])
                stq[i] = (x1, qT)

            def emit_mm(i):
                """up-proj matmuls + GLU + act-quant + gqT for tile i."""
                x1, qT = stq[i]
                g = gpool.tile([128, F], dt.float32, tag="g")
                gm8 = sc.tile([128, NFC], dt.float32, tag="gm8")
                for f in range(NFC):
                    ch = f // 2
                    fo = (f % 2) * 512
                    u_ps = ps_up.tile([128, 512], dt.float32, tag="ups")
                    v_ps = ps_up.tile([128, 512], dt.float32, tag="ups")
                    for dk in range(NDK):
                        nc.tensor.matmul(
                            u_ps[:], qT[:, dk, :], wg_c[ch][:, dk, fo:fo + 512],
                            start=(dk == 0), stop=(dk == NDK - 1),
                        )
                        nc.tensor.matmul(
                            v_ps[:], qT[:, dk, :], wu_c[ch][:, dk, fo:fo + 512],
                            start=(dk == 0), stop=(dk == NDK - 1),
                        )
                    su = silupool.tile([128, 512], dt.float32, tag="silu")
                    nc.scalar.activation(su[:], u_ps[:], AF.Silu,
                                         scale=rs_all[:, i:i + 1])
                    nc.vector.tensor_tensor(
                        g[:, f * 512:(f + 1) * 512], su[:], v_ps[:], op=ALU.mult
                    )
                    nc.vector.tensor_reduce(
                        gm8[:, f:f + 1], g[:, f * 512:(f + 1) * 512],
                        axis=mybir.AxisListType.X, op=ALU.max, apply_absolute_value=True,
                    )
                # s2 = clip(127/(max|g|/s + 1e-5)); c2 = s2/s ; rs2 = 1/s2
                gmx = sc.tile([128, 1], dt.float32, tag="gmx")
                nc.vector.tensor_reduce(
                    gmx[:], gm8[:], axis=mybir.AxisListType.X, op=ALU.max,
                    apply_absolute_value=False,
                )
                nc.vector.tensor_tensor(gmx[:], gmx[:], rs_all[:, i:i + 1], op=ALU.mult)
                nc.vector.tensor_scalar(gmx[:], gmx[:], 1e-5, None, op0=ALU.add)
                s2r = sc.tile([128, 1], dt.float32, tag="s2r")
                nc.vector.reciprocal(s2r[:], gmx[:])
                s2 = sc.tile([128, 1], dt.float32, tag="s2")
                nc.vector.tensor_scalar(s2[:], s2r[:], 127.0, 1e3, op0=ALU.mult, op1=ALU.min)
                nc.vector.tensor_scalar(s2[:], s2[:], 1e-3, None, op0=ALU.max)
                c2 = sc.tile([128, 1], dt.float32, tag="c2")
                nc.vector.tensor_tensor(c2[:], s2[:], rs_all[:, i:i + 1], op=ALU.mult)
                rs2 = sc.tile([128, 1], dt.float32, tag="rs2")
                nc.vector.reciprocal(rs2[:], s2[:])
                # gq = round(c2*g) in two halves (ACT magic pass in place, DVE
                # subtract to bf16, transpose) so the serial tail is half as long
                gq = gqpool.tile([128, F], dt.bfloat16, tag="gq")
                gqT = gqtpool.tile([128, NFT, 128], dt.bfloat16, tag="gqt")
                for h in range(2):
                    hs = h * (F // 2)
                    he = hs + F // 2
                    nc.scalar.activation(g[:, hs:he], g[:, hs:he], AF.Identity,
                                         scale=c2[:], bias=mag[:])
                    nc.vector.tensor_scalar(gq[:, hs:he], g[:, hs:he], MAGIC, None,
                                            op0=ALU.subtract)
                    nc.scalar.dma_start_transpose(
                        gqT[:, h * (NFT // 2):(h + 1) * (NFT // 2), :], gq[:, hs:he])
                std[i] = (x1, gqT, rs2)
                stq[i] = None

            def emit_down(i):
                x1, gqT, rs2 = std[i]
                xd_ps0 = ps_dn.tile([128, 512], dt.float32, tag="dps")
                xd_ps1 = ps_dn.tile([128, 512], dt.float32, tag="dps")
                for ft in range(NFT):
                    nc.tensor.matmul(
                        xd_ps0[:], gqT[:, ft, :], wd_t[:, ft, 0:512],
                        start=(ft == 0), stop=(ft == NFT - 1),
                    )
                    nc.tensor.matmul(
                        xd_ps1[:], gqT[:, ft, :], wd_t[:, ft, 512:1024],
                        start=(ft == 0), stop=(ft == NFT - 1),
                    )
                xnew = s4k.tile([128, D], dt.float32, tag="s4k")
                for dc, xd_ps in ((0, xd_ps0), (1, xd_ps1)):
                    xdr = xdrpool.tile([128, 512], dt.float32, tag="xdr")
                    nc.scalar.activation(xdr[:], xd_ps[:], AF.Copy, scale=rs2[:])
                    nc.vector.tensor_tensor(
                        xnew[:, dc * 512:(dc + 1) * 512],
                        x1[:, dc * 512:(dc + 1) * 512], xdr[:], op=ALU.add,
                    )
                if l + 1 < n_layers:
                    tile_stats(xnew[:], ssq_n, mx_n, i, scale_bc_next)
                    if i == 3:
                        stats_chain(ssq_n, mx_n, c1_n, rs_n, 0, 4)
                    elif i == n_tok_tiles - 1:
                        stats_chain(ssq_n, mx_n, c1_n, rs_n, 4, n_tok_tiles)
                nc.scalar.dma_start(xdst[i * 128:(i + 1) * 128, :], xnew[:])
                std[i] = None

            # software pipeline with one-tile lookahead on q/qT:
            # q0 q1 | mm0 q2 | mm1 q3 dn0 | mm2 q4 dn1 | ... | mm7 dn6 | dn7
            emit_q(0)
            emit_q(1)
            for i in range(n_tok_tiles):
                emit_mm(i)
                if i + 2 < n_tok_tiles:
                    emit_q(i + 2)
                if i >= 1:
                    emit_down(i - 1)
                if i == 2 and l + 1 < n_layers:
                    coop_prefetch(l + 1)
            emit_down(n_tok_tiles - 1)

            xsrc = xdst

    _split_excess_waits(nc)
    return nc


_nc_cache = {}


def _get_nc(key):
    if key not in _nc_cache:
        _nc_cache[key] = build(*key)
    return _nc_cache[key]


def _repack(w, n_layers):
    # [L, D, F] -> [L, F//1024, D//128, 128, 1024] so each (ch, dk) tile is
    # contiguous; dk == core index for the per-core max slices.
    return np.ascontiguousarray(
        w.reshape(n_layers, D // 128, 128, NCH, 1024).transpose(0, 3, 1, 2, 4)
    )


def _make_in_maps(x, rs, wg, wu, wd, n_cores=NCORES):
    n_layers = rs.shape[0]
    wg_r = _repack(wg, n_layers)
    wu_r = _repack(wu, n_layers)
    nsl = NDK // n_cores  # dk-slabs per core for slg/slu
    dsl = F // n_cores
    in_maps = []
    for c in range(n_cores):
        slg = wg_r[:, :, c * nsl:(c + 1) * nsl].reshape(n_layers, -1, 128, 1024)
        slu = wu_r[:, :, c * nsl:(c + 1) * nsl].reshape(n_layers, -1, 128, 1024)
        sld = wd[:, c * dsl:(c + 1) * dsl, :].reshape(n_layers, -1, 128, 1024)
        in_maps.append({
            "x": x[c],
            "rs": rs,
            "wg": wg_r,
            "wu": wu_r,
            "wd": wd,
            "slg": np.ascontiguousarray(slg),
            "slu": np.ascontiguousarray(slu),
            "sld": np.ascontiguousarray(sld),
        })
    return in_maps


def kernel(x, rms_scale, W_g, W_u, W_d):
    """Full-input entry point: shard over batch, run 8-core SPMD, gather."""
    x = np.ascontiguousarray(np.asarray(x, dtype=np.float32))
    rs = np.ascontiguousarray(np.asarray(rms_scale, dtype=np.float32))
    wg = np.ascontiguousarray(np.asarray(W_g, dtype=np.float32))
    wu = np.ascontiguousarray(np.asarray(W_u, dtype=np.float32))
    wd = np.ascontiguousarray(np.asarray(W_d, dtype=np.float32))
    B, S, Dx = x.shape
    assert (B, S, Dx) == (NCORES, TOK, D), (B, S, Dx)
    is_ones = bool(np.all(rs == 1.0))
    nc = _get_nc((is_ones,))
    in_maps = _make_in_maps(x, rs, wg, wu, wd)
    res = run_bass_kernel_spmd(nc, in_maps, list(range(NCORES)))
    return np.stack([res.results[c]["out"] for c in range(NCORES)], axis=0)


# revision 20
# speedup vs baseline: 1.2692x; 1.0488x over previous
"""Trainium2 Bass kernel for nn_CascadeTransformerMM (4-layer ternary-GLU cascade).

Math (per layer, per token row):
  h   = rms_scale * x * rsqrt(mean(x^2) + 1e-6)
  s   = clip(127/(max|h| + 1e-5), 1e-3, 1e3);  q = round(s*h)      (ints in [-127,127])
  Wt  = clip(round(W * 127/(max|W| + 1e-5)), -1, 1)                 (ternary {-1,0,1})
  u   = (q @ Wg_t)/s ; v = (q @ Wu_t)/s ; g = silu(u)*v
  s2  = clip(127/(max|g| + 1e-5), 1e-3, 1e3); gq = round(s2*g)
  x  += (gq @ Wd_t)/s2

Distribution: pure data-parallel over the batch dim (8 batches -> 8 cores),
weights replicated per core. Per-matrix |W|max is computed cooperatively:
each core reduces a 1/8 row-slice, then a tiny AllReduce(max) shares the 12
scalars (layer 0's three matrices allreduced first so its ternarize can
start early). All matmuls run on the PE array with bf16 activations
(integers <= 127, exact) x fp8 ternary weights (exact), fp32 PSUM
accumulation -> the heavy compute is bit-exact integer arithmetic.

Schedule: per-token-tile software pipeline with one-tile lookahead --
q/qT production for tile i+2 and down-proj for tile i-1 are emitted
around the up-proj matmul block of tile i, so the serial per-tile tail
(s2 chain -> gq round -> gqT transpose) hides under the next tile's
matmuls and the PE stream stays dense (keeps the HAM clock-gate warm).
Weights live in per-column-chunk SBUF tiles so layer-0 ternarize and
layer-boundary refills unblock the matmul stream progressively.
Ternarize runs pass 1 (scale+round-to-int32) on DVE and pass 2 (clip to
{-1,0,1} as fp8) on the otherwise-idle GPSIMD (alternating with ACT
Sign for layer 0, where latency matters)."""

import os
import sys

for _p in ("/opt/trn_rl_repo", "/root/.axon_site/_ro/trn_rl_repo"):
    if os.path.isdir(_p) and _p not in sys.path:
        sys.path.insert(0, _p)

import numpy as np
from contextlib import ExitStack

import concourse.bass as bass
import concourse.bass_isa as bass_isa
import concourse.mybir as mybir
import concourse.tile as tile
from concourse.bass_utils import run_bass_kernel_spmd

dt = mybir.dt
AF = mybir.ActivationFunctionType
ALU = mybir.AluOpType

MAGIC = float(1.5 * 2**23)  # fp32 round-to-nearest-even magic constant
D = 1024
F = 4096
L = 4
NCORES = 8
TOK = 1024  # tokens per core (one batch of S=1024)

NDK = D // 128   # 8 contraction tiles for up-proj
NFT = F // 128   # 32 contraction tiles for down-proj
NFC = F // 512   # 8 free-dim chunks for up-proj
NCH = F // 1024  # 4 column-chunks in the repacked wg/wu layout
NTT = TOK // 128  # 8 token tiles


def _split_excess_waits(nc, max_waits: int = 1) -> int:
    """walrus in this container rejects >1 sync-wait per instruction; split
    extras into standalone event-semaphore waits on the same engine (same-
    engine program order makes this semantically identical)."""
    n = 0
    for func in nc.m.functions:
        for block in func.blocks:
            changed = False
            out = []
            for inst in block.instructions:
                si = getattr(inst, "sync_info", None)
                if si is not None and si.on_wait and len(si.on_wait) > max_waits:
                    waits = list(si.on_wait)
                    for j, w in enumerate(waits[max_waits:]):
                        out.append(
                            mybir.InstEventSemaphore(
                                name=f"{inst.name}-xw{j}",
                                engine=inst.engine,
                                ins=[],
                                outs=[],
                                sync_info=mybir.SyncInfo(on_wait=[w], on_update=[]),
                            )
                        )
                        n += 1
                    inst.sync_info = mybir.SyncInfo(
                        on_wait=waits[:max_waits], on_update=list(si.on_update)
                    )
                    changed = True
                out.append(inst)
            if changed:
                block.instructions = out
    return n


def build(is_ones: bool = True, n_cores: int = NCORES, n_tok_tiles: int = NTT,
          n_layers: int = L) -> bass.Bass:
    nc = bass.Bass(num_devices=n_cores)
    tok = n_tok_tiles * 128

    x_ext = nc.declare_dram_parameter("x", [tok, D], dt.float32, isOutput=False)
    rs_ext = nc.declare_dram_parameter("rs", [n_layers, D], dt.float32, isOutput=False)
    # wg/wu repacked host-side to [L, F//1024, NDK, 128, 1024] so every
    # [128, 1024] weight tile is one contiguous 512 KB DMA; wd's row-slabs
    # are naturally contiguous.
    wg_ext = nc.declare_dram_parameter("wg", [n_layers, NCH, NDK, 128, 1024], dt.float32, isOutput=False)
    wu_ext = nc.declare_dram_parameter("wu", [n_layers, NCH, NDK, 128, 1024], dt.float32, isOutput=False)
    wd_ext = nc.declare_dram_parameter("wd", [n_layers, F, D], dt.float32, isOutput=False)
    # per-core row-slices of each matrix for the cooperative |W|max,
    # as [L, nun, 128, 1024] contiguous units
    nun = (D // n_cores) * F // (128 * 1024)
    slg_ext = nc.declare_dram_parameter("slg", [n_layers, nun, 128, 1024], dt.float32, isOutput=False)
    slu_ext = nc.declare_dram_parameter("slu", [n_layers, nun, 128, 1024], dt.float32, isOutput=False)
    sld_ext = nc.declare_dram_parameter("sld", [n_layers, nun, 128, 1024], dt.float32, isOutput=False)
    out_ext = nc.declare_dram_parameter("out", [tok, D], dt.float32, isOutput=True)

    mx_locs = [nc.dram_tensor(f"mx{l}_loc", [1, 16], dt.float32) for l in range(n_layers)]
    mx_globs = [nc.dram_tensor(f"mx{l}_glob", [1, 16], dt.float32) for l in range(n_layers)]
    # AllGather outputs for the cooperative ternarize, split per column-chunk
    # so downstream fills (and layer 0's first matmuls) gate on 2 MB gathers
    # instead of the full 12.6 MB: gac[l][ch] = [core, 128, 2048] (wg|wu cols),
    # gac[l][4] = [core, 128, 4096] (wd's 4 ft units per core).
    gac = []
    for _l in range(n_layers):
        row = [
            nc.dram_tensor(f"gac{_l}_{k}", [n_cores, 128, 2048], dt.float8e4,
                           addr_space="Shared")
            for k in range(NCH)
        ]
        row.append(nc.dram_tensor(f"gac{_l}_d", [n_cores, 128, 4096], dt.float8e4,
                                  addr_space="Shared"))
        gac.append(row)

    with tile.TileContext(nc) as tc, ExitStack() as ctx:
        P = ctx.enter_context
        wch = P(tc.tile_pool(name="wch", bufs=2 * NCH))   # per-ch wg/wu fp8 tiles
        wdp = P(tc.tile_pool(name="wdp", bufs=1))         # wd fp8 tile
        wstream = P(tc.tile_pool(name="wstream", bufs=2))
        wi32 = P(tc.tile_pool(name="wi32", bufs=2))
        xpool = P(tc.tile_pool(name="x1", bufs=5))
        t1pool = P(tc.tile_pool(name="t1", bufs=1))
        s4k = P(tc.tile_pool(name="s4k", bufs=2))         # xnew scratch
        qpool = P(tc.tile_pool(name="q", bufs=2))
        qtpool = P(tc.tile_pool(name="qt", bufs=2))
        silupool = P(tc.tile_pool(name="silu", bufs=2))   # [128,512] silu chunks
        scrpool = P(tc.tile_pool(name="scr", bufs=1))     # square scratch
        gpool = P(tc.tile_pool(name="g", bufs=1))
        gqpool = P(tc.tile_pool(name="gq", bufs=1))
        gqtpool = P(tc.tile_pool(name="gqt", bufs=2))
        w8s = P(tc.tile_pool(name="w8s", bufs=1))
        batch = P(tc.tile_pool(name="batch", bufs=2))     # [128, ntt] per-layer stats
        btmp = P(tc.tile_pool(name="btmp", bufs=1))       # stats chain temps
        sc = P(tc.tile_pool(name="sc", bufs=4))           # [128, small] scalars
        xdrpool = P(tc.tile_pool(name="xdr", bufs=1))
        const = P(tc.tile_pool(name="const", bufs=1))
        bcpool = P(tc.tile_pool(name="bc", bufs=2))       # rms_scale broadcast (general path)
        dram = P(tc.tile_pool(name="dram", bufs=2, space="DRAM"))
        ps_up = P(tc.tile_pool(name="psup", bufs=4, space="PSUM"))
        ps_dn = P(tc.tile_pool(name="psdn", bufs=4, space="PSUM"))

        # ---------- constants ----------
        mag = const.tile([128, 1], dt.float32, tag="mag")
        nc.gpsimd.memset(mag[:], MAGIC)
        ones1 = const.tile([1, 128], dt.float32, tag="ones1")
        nc.gpsimd.memset(ones1[:], 1.0)

        def pe_broadcast(dst, src_row, n):
            """broadcast src_row [1, n] to dst [128, n] via PE outer product."""
            for h in range(0, n, 512):
                w = min(512, n - h)
                bc_ps = ps_up.tile([128, 512], dt.float32, tag="ups")
                nc.tensor.matmul(bc_ps[:, 0:w], ones1[:], src_row[:, h:h + w],
                                 start=True, stop=True)
                nc.scalar.activation(dst[:, h:h + w], bc_ps[:, 0:w], AF.Copy)

        # ---------- cooperative per-matrix |W|max ----------
        # Layer 0's three matrices are reduced + allreduced first so its
        # ternarize can start while layers 1-3 slices still stream.
        wmax_cols = const.tile([128, 16], dt.float32, tag="wmaxc")
        nc.gpsimd.memset(wmax_cols[:], 0.0)

        def slab_max(l, ext, idx, eng):
            part = sc.tile([128, nun], dt.float32, tag="wmaxpart")
            for u in range(nun):
                wt = wstream.tile([128, 1024], dt.float32, tag="wstream")
                eng.dma_start(wt[:], ext[l, u])
                nc.vector.tensor_reduce(
                    part[:, u:u + 1], wt[:], axis=mybir.AxisListType.X,
                    op=ALU.max, apply_absolute_value=True,
                )
            nc.vector.tensor_reduce(
                wmax_cols[:, idx:idx + 1], part[:, 0:nun],
                axis=mybir.AxisListType.X, op=ALU.max,
                apply_absolute_value=False,
            )

        wscl = []
        for _l in range(n_layers):
            wsc_t = const.tile([128, 16], dt.float32, tag=f"wsc{_l}")
            wscl.append(wsc_t)
        grow = const.tile([1, 16], dt.float32, tag="grow")
        mrow = const.tile([1, 16], dt.float32, tag="mrow")
        nc.gpsimd.memset(mrow[:], 0.0)
        wsc_row = const.tile([1, 16], dt.float32, tag="wscrow")

        def finish_wsc(cols, loc, glob, wsc):
            """partition-reduce wmax cols, allreduce across cores, build
            wsc[:, cols] = 127/(m+1e-5) broadcast to all partitions."""
            a, b = cols
            nc.gpsimd.tensor_reduce(
                mrow[:, a:b], wmax_cols[:, a:b], axis=mybir.AxisListType.C, op=ALU.max
            )
            nc.scalar.dma_start(loc[:], mrow[:])
            nc.gpsimd.collective_compute(
                "AllReduce",
                ALU.max,
                replica_groups=[list(range(n_cores))],
                ins=[loc[:].opt()],
                outs=[glob[:].opt()],
            )
            nc.scalar.dma_start(grow[:, a:b], glob[:, a:b])
            nc.vector.tensor_scalar(wsc_row[:, a:b], grow[:, a:b], 1e-5, None, op0=ALU.add)
            nc.vector.reciprocal(wsc_row[:, a:b], wsc_row[:, a:b])
            nc.vector.tensor_scalar(wsc_row[:, a:b], wsc_row[:, a:b], 127.0, None, op0=ALU.mult)
            pe_broadcast(wsc, wsc_row, 16)

        # ---------- ternarize ----------
        def tern_unit(src, dst_ap, idx, pass2_eng="gp"):
            """src: [128, 1024] fp32 AP; dst_ap: fp8 [128, 1024]-shaped."""
            wt = wstream.tile([128, 1024], dt.float32, tag="wstream")
            nc.sync.dma_start(wt[:], src)
            r32 = wi32.tile([128, 1024], dt.int32, tag="wi32")
            wsc = wscl[idx // 3]
            nc.vector.tensor_scalar(
                r32[:], wt[:], wsc[:, idx:idx + 1], None, op0=ALU.mult
            )
            if pass2_eng == "act":
                # sign(n) == clip(n, -1, 1) for integer n
                nc.scalar.activation(dst_ap, r32[:], AF.Sign)
            else:
                nc.gpsimd.tensor_scalar(dst_ap, r32[:], 1, -1, op0=ALU.min, op1=ALU.max)

        # ---------- cooperative weight prefetch (layers >= 1) ----------
        # Each core reduces + ternarizes only its 1/8 shard (the same
        # host-staged slices used for the |W|max pass), then one fp8
        # AllGather per layer shares the ternary weights; the gather hides
        # under the previous layer's compute.
        def coop_prefetch(lp):
            for mi, ext in enumerate((slg_ext, slu_ext, sld_ext)):
                slab_max(lp, ext, 3 * lp + mi, nc.sync)
            finish_wsc((3 * lp, 3 * lp + 3), mx_locs[lp], mx_globs[lp], wscl[lp])
            def gather(in_ap, out_ap):
                nc.gpsimd.collective_compute(
                    "AllGather",
                    ALU.bypass,
                    replica_groups=[list(range(n_cores))],
                    ins=[in_ap.opt()],
                    outs=[out_ap.opt()],
                )

            alt = [0]

            def tern_to(src, dst, idx):
                stg = w8s.tile([128, 1024], dt.float8e4, tag="w8s")
                tern_unit(src, stg[:], idx, "act" if alt[0] % 2 == 0 else "gp")
                alt[0] += 1
                nc.sync.dma_start(dst, stg[:])

            for u in range(nun):
                lck = dram.tile([128, 2048], dt.float8e4, tag=f"locc{u}")
                tern_to(slg_ext[lp, u], lck[:, 0:1024], 3 * lp)
                tern_to(slu_ext[lp, u], lck[:, 1024:2048], 3 * lp + 1)
                gather(lck[:], gac[lp][u][:])
            lcd = dram.tile([128, 4096], dt.float8e4, tag="locd")
            for u in range(nun):
                tern_to(sld_ext[lp, u], lcd[:, u * 1024:(u + 1) * 1024], 3 * lp + 2)
            gather(lcd[:], gac[lp][4][:])

        # ---------- per-layer stats ----------
        # ssq/mx filled per token tile; the scalar chain runs on column
        # groups [0:4] and [4:8] as soon as those tiles' stats exist.
        def stats_chain(ssq_all, mx_all, c1_all, rs_all, a, b):
            k = b - a
            ms_t = btmp.tile([128, n_tok_tiles], dt.float32, tag="ms")
            rt_t = btmp.tile([128, n_tok_tiles], dt.float32, tag="rt")
            rstd_t = btmp.tile([128, n_tok_tiles], dt.float32, tag="rstd")
            nwt_t = btmp.tile([128, n_tok_tiles], dt.float32, tag="nwt")
            maxh_t = btmp.tile([128, n_tok_tiles], dt.float32, tag="maxh")
            sr_t = btmp.tile([128, n_tok_tiles], dt.float32, tag="sr")
            s_t = btmp.tile([128, n_tok_tiles], dt.float32, tag="s_")
            ms, rt, rstd = ms_t[:, 0:k], rt_t[:, 0:k], rstd_t[:, 0:k]
            nwt, maxh, sr, s_ = nwt_t[:, 0:k], maxh_t[:, 0:k], sr_t[:, 0:k], s_t[:, 0:k]
            nc.vector.tensor_scalar(ms, ssq_all[:, a:b], 1.0 / D, 1e-6, op0=ALU.mult, op1=ALU.add)
            nc.scalar.activation(rt, ms, AF.Sqrt)
            nc.vector.reciprocal(rstd, rt)
            # one Newton step: rstd *= 1.5 - 0.5*ms*rstd^2  (fixes the ~7e-6
            # Sqrt-LUT error that quantization tie-flips amplify layer by layer)
            nc.vector.tensor_tensor(nwt, rstd, rstd, op=ALU.mult)
            nc.vector.tensor_tensor(nwt, nwt, ms, op=ALU.mult)
            nc.vector.tensor_scalar(nwt, nwt, -0.5, 1.5, op0=ALU.mult, op1=ALU.add)
            nc.vector.tensor_tensor(rstd, rstd, nwt, op=ALU.mult)
            nc.vector.tensor_tensor(maxh, mx_all[:, a:b], rstd, op=ALU.mult)
            nc.vector.tensor_scalar(maxh, maxh, 1e-5, None, op0=ALU.add)
            nc.vector.reciprocal(sr, maxh)
            nc.vector.tensor_scalar(s_, sr, 127.0, 1e3, op0=ALU.mult, op1=ALU.min)
            nc.vector.tensor_scalar(s_, s_, 1e-3, None, op0=ALU.max)
            nc.vector.tensor_tensor(c1_all[:, a:b], s_, rstd, op=ALU.mult)
            nc.vector.reciprocal(rs_all[:, a:b], s_)

        def tile_stats(src_ap, ssq_all, mx_all, i, scale_bc):
            """ssq + scaled abs-max for token tile i of the NEXT layer's input."""
            scr = scrpool.tile([128, D], dt.float32, tag="scr")
            nc.scalar.activation(scr[:], src_ap, AF.Square, accum_out=ssq_all[:, i:i + 1])
            if is_ones:
                nc.vector.tensor_reduce(
                    mx_all[:, i:i + 1], src_ap, axis=mybir.AxisListType.X,
                    op=ALU.max, apply_absolute_value=True,
                )
            else:
                nc.vector.tensor_tensor(scr[:], src_ap, scale_bc[:], op=ALU.mult)
                nc.vector.tensor_reduce(
                    mx_all[:, i:i + 1], scr[:], axis=mybir.AxisListType.X,
                    op=ALU.max, apply_absolute_value=True,
                )

        def do_q(xsrc_, c1_t, sc_bc, stq_, i):
            """load x tile i, quantize with c1_t[:, i], transpose."""
            x1 = xpool.tile([128, D], dt.float32, tag="x1")
            nc.scalar.dma_start(x1[:], xsrc_[i * 128:(i + 1) * 128, :])
            t1 = t1pool.tile([128, D], dt.float32, tag="t1")
            # q = round(c1 * h') via magic add (ACT) / subtract (DVE), out bf16
            if is_ones:
                nc.scalar.activation(t1[:], x1[:], AF.Identity,
                                     scale=c1_t[:, i:i + 1], bias=mag[:])
            else:
                nc.vector.tensor_tensor(t1[:], x1[:], sc_bc[:], op=ALU.mult)
                nc.scalar.activation(t1[:], t1[:], AF.Identity,
                                     scale=c1_t[:, i:i + 1], bias=mag[:])
            q = qpool.tile([128, D], dt.bfloat16, tag="q")
            nc.vector.tensor_scalar(q[:], t1[:], MAGIC, None, op0=ALU.subtract)
            qT = qtpool.tile([128, NDK, 128], dt.bfloat16, tag="qt")
            nc.scalar.dma_start_transpose(qT[:], q[:])
            stq_[i] = (x1, qT)

        # layer 0's weights: cooperative ternarize + gather up front
        coop_prefetch(0)

        # ---------- layers ----------
        xsrc = x_ext
        prev_stats = None  # (ssq, mx, c1, rs) for current layer, from fused pass
        carry_stq = None   # q/qT for tiles 0-1, pre-emitted by the previous layer

        for l in range(n_layers):
            wg_c, wu_c = [], []
            for _ch in range(NCH):
                wgc_t = wch.tile([128, NDK, 1024], dt.float8e4, tag="wch")
                wg_c.append(wgc_t)
            for _ch in range(NCH):
                wuc_t = wch.tile([128, NDK, 1024], dt.float8e4, tag="wch")
                wu_c.append(wuc_t)
            wd_t = wdp.tile([128, NFT, D], dt.float8e4, tag="wdp")
            # fill in consumption order: wg/wu chunk-columns, then wd
            for ch in range(NCH):
                nc.sync.dma_start(
                    wg_c[ch][:],
                    gac[l][ch][:, :, 0:1024].transpose([1, 0, 2]))
                nc.sync.dma_start(
                    wu_c[ch][:],
                    gac[l][ch][:, :, 1024:2048].transpose([1, 0, 2]))
            for c in range(n_cores):
                nc.sync.dma_start(wd_t[:, 4 * c:4 * c + 4, :], gac[l][4][c])

            scale_bc = None
            scale_bc_next = None
            if not is_ones:
                rs_row = const.tile([1, D], dt.float32, tag=f"rsrow{l}")
                nc.sync.dma_start(rs_row[:], rs_ext[l:l + 1, :])
                scale_bc = bcpool.tile([128, D], dt.float32, tag="bc")
                pe_broadcast(scale_bc, rs_row[:], D)
                if l + 1 < n_layers:
                    rs_row_n = const.tile([1, D], dt.float32, tag=f"rsrown{l}")
                    nc.sync.dma_start(rs_row_n[:], rs_ext[l + 1:l + 2, :])
                    scale_bc_next = bcpool.tile([128, D], dt.float32, tag="bcn")
                    pe_broadcast(scale_bc_next, rs_row_n[:], D)

            if l == 0:
                # phase A for layer 0 only: standalone stats pass
                ssq_all = batch.tile([128, n_tok_tiles], dt.float32, tag="ssq")
                mx_all = batch.tile([128, n_tok_tiles], dt.float32, tag="mx")
                c1_all = batch.tile([128, n_tok_tiles], dt.float32, tag="c1")
                rs_all = batch.tile([128, n_tok_tiles], dt.float32, tag="rs_all")
                for i in range(n_tok_tiles):
                    xa = xpool.tile([128, D], dt.float32, tag="x1")
                    nc.scalar.dma_start(xa[:], xsrc[i * 128:(i + 1) * 128, :])
                    tile_stats(xa[:], ssq_all, mx_all, i, scale_bc)
                    if i == 3:
                        stats_chain(ssq_all, mx_all, c1_all, rs_all, 0, 4)
                    elif i == n_tok_tiles - 1:
                        stats_chain(ssq_all, mx_all, c1_all, rs_all, 4, n_tok_tiles)
            else:
                ssq_all, mx_all, c1_all, rs_all = prev_stats

            if l + 1 < n_layers:
                ssq_n = batch.tile([128, n_tok_tiles], dt.float32, tag="ssq")
                mx_n = batch.tile([128, n_tok_tiles], dt.float32, tag="mx")
                c1_n = batch.tile([128, n_tok_tiles], dt.float32, tag="c1")
                rs_n = batch.tile([128, n_tok_tiles], dt.float32, tag="rs_all")
                prev_stats = (ssq_n, mx_n, c1_n, rs_n)

            xdst = out_ext if l == n_layers - 1 else dram.tile([tok, D], dt.float32, tag="xbuf")

            # per-tile state
            stq = carry_stq if carry_stq is not None else [None] * n_tok_tiles
            carry_stq = None
            std = [None] * n_tok_tiles   # (x1, gqT, rs2) from emit_mm

            def emit_q(i):
                do_q(xsrc, c1_all, scale_bc, stq, i)

            def emit_mm(i):
                """up-proj matmuls + GLU + act-quant + gqT for tile i."""
                x1, qT = stq[i]
                g = gpool.tile([128, F], dt.float32, tag="g")
                gm8 = sc.tile([128, NFC], dt.float32, tag="gm8")
                for f in range(NFC):
                    ch = f // 2
                    fo = (f % 2) * 512
                    u_ps = ps_up.tile([128, 512], dt.float32, tag="ups")
                    v_ps = ps_up.tile([128, 512], dt.float32, tag="ups")
                    for dk in range(NDK):
                        nc.tensor.matmul(
                            u_ps[:], qT[:, dk, :], wg_c[ch][:, dk, fo:fo + 512],
                            start=(dk == 0), stop=(dk == NDK - 1),
                        )
                        nc.tensor.matmul(
                            v_ps[:], qT[:, dk, :], wu_c[ch][:, dk, fo:fo + 512],
                            start=(dk == 0), stop=(dk == NDK - 1),
                        )
                    su = silupool.tile([128, 512], dt.float32, tag="silu")
                    nc.scalar.activation(su[:], u_ps[:], AF.Silu,
                                         scale=rs_all[:, i:i + 1])
                    nc.vector.tensor_tensor(
                        g[:, f * 512:(f + 1) * 512], su[:], v_ps[:], op=ALU.mult
                    )
                    nc.vector.tensor_reduce(
                        gm8[:, f:f + 1], g[:, f * 512:(f + 1) * 512],
                        axis=mybir.AxisListType.X, op=ALU.max, apply_absolute_value=True,
                    )
                # s2 = clip(127/(max|g|/s + 1e-5)); c2 = s2/s ; rs2 = 1/s2
                gmx = sc.tile([128, 1], dt.float32, tag="gmx")
                nc.vector.tensor_reduce(
                    gmx[:], gm8[:], axis=mybir.AxisListType.X, op=ALU.max,
                    apply_absolute_value=False,
                )
                nc.vector.tensor_tensor(gmx[:], gmx[:], rs_all[:, i:i + 1], op=ALU.mult)
                nc.vector.tensor_scalar(gmx[:], gmx[:], 1e-5, None, op0=ALU.add)
                s2r = sc.tile([128, 1], dt.float32, tag="s2r")
                nc.vector.reciprocal(s2r[:], gmx[:])
                s2 = sc.tile([128, 1], dt.float32, tag="s2")
                nc.vector.tensor_scalar(s2[:], s2r[:], 127.0, 1e3, op0=ALU.mult, op1=ALU.min)
                nc.vector.tensor_scalar(s2[:], s2[:], 1e-3, None, op0=ALU.max)
                c2 = sc.tile([128, 1], dt.float32, tag="c2")
                nc.vector.tensor_tensor(c2[:], s2[:], rs_all[:, i:i + 1], op=ALU.mult)
                rs2 = sc.tile([128, 1], dt.float32, tag="rs2")
                nc.vector.reciprocal(rs2[:], s2[:])
                # gq = round(c2*g) in two halves (ACT magic pass in place, DVE
                # subtract to bf16, transpose) so the serial tail is half as long
                gq = gqpool.tile([128, F], dt.bfloat16, tag="gq")
                gqT = gqtpool.tile([128, NFT, 128], dt.bfloat16, tag="gqt")
                for h in range(2):
                    hs = h * (F // 2)
                    he = hs + F // 2
                    nc.scalar.activation(g[:, hs:he], g[:, hs:he], AF.Identity,
                                         scale=c2[:], bias=mag[:])
                    nc.vector.tensor_scalar(gq[:, hs:he], g[:, hs:he], MAGIC, None,
                                            op0=ALU.subtract)
                    nc.scalar.dma_start_transpose(
                        gqT[:, h * (NFT // 2):(h + 1) * (NFT // 2), :], gq[:, hs:he])
                std[i] = (x1, gqT, rs2)
                stq[i] = None

            def emit_down(i):
                x1, gqT, rs2 = std[i]
                xd_ps0 = ps_dn.tile([128, 512], dt.float32, tag="dps")
                xd_ps1 = ps_dn.tile([128, 512], dt.float32, tag="dps")
                for ft in range(NFT):
                    nc.tensor.matmul(
                        xd_ps0[:], gqT[:, ft, :], wd_t[:, ft, 0:512],
                        start=(ft == 0), stop=(ft == NFT - 1),
                    )
                    nc.tensor.matmul(
                        xd_ps1[:], gqT[:, ft, :], wd_t[:, ft, 512:1024],
                        start=(ft == 0), stop=(ft == NFT - 1),
                    )
                xnew = s4k.tile([128, D], dt.float32, tag="s4k")
                for dc, xd_ps in ((0, xd_ps0), (1, xd_ps1)):
                    xdr = xdrpool.tile([128, 512], dt.float32, tag="xdr")
                    nc.scalar.activation(xdr[:], xd_ps[:], AF.Copy, scale=rs2[:])
                    nc.vector.tensor_tensor(
                        xnew[:, dc * 512:(dc + 1) * 512],
                        x1[:, dc * 512:(dc + 1) * 512], xdr[:], op=ALU.add,
                    )
                if l + 1 < n_layers:
                    tile_stats(xnew[:], ssq_n, mx_n, i, scale_bc_next)
                    if i == 3:
                        stats_chain(ssq_n, mx_n, c1_n, rs_n, 0, 4)
                    elif i == n_tok_tiles - 1:
                        stats_chain(ssq_n, mx_n, c1_n, rs_n, 4, n_tok_tiles)
                nc.scalar.dma_start(xdst[i * 128:(i + 1) * 128, :], xnew[:])
                std[i] = None

            # software pipeline with one-tile lookahead on q/qT:
            # q0 q1 | mm0 q2 | mm1 q3 dn0 | mm2 q4 dn1 | ... | mm7 dn6 | q'0 q'1 dn7
            if stq[0] is None:
                emit_q(0)
                emit_q(1)
            for i in range(n_tok_tiles):
                emit_mm(i)
                if i + 2 < n_tok_tiles:
                    emit_q(i + 2)
                if i >= 1:
                    emit_down(i - 1)
                if i == 2 and l + 1 < n_layers:
                    coop_prefetch(l + 1)
            if l + 1 < n_layers:
                carry_stq = [None] * n_tok_tiles
                do_q(xdst, c1_n, scale_bc_next, carry_stq, 0)
                do_q(xdst, c1_n, scale_bc_next, carry_stq, 1)
            emit_down(n_tok_tiles - 1)

            xsrc = xdst

    _split_excess_waits(nc)
    return nc


_nc_cache = {}


def _get_nc(key):
    if key not in _nc_cache:
        _nc_cache[key] = build(*key)
    return _nc_cache[key]


def _repack(w, n_layers):
    # [L, D, F] -> [L, F//1024, D//128, 128, 1024] so each (ch, dk) tile is
    # contiguous; dk == core index for the per-core max slices.
    return np.ascontiguousarray(
        w.reshape(n_layers, D // 128, 128, NCH, 1024).transpose(0, 3, 1, 2, 4)
    )


def _make_in_maps(x, rs, wg, wu, wd, n_cores=NCORES):
    n_layers = rs.shape[0]
    wg_r = _repack(wg, n_layers)
    wu_r = _repack(wu, n_layers)
    nsl = NDK // n_cores  # dk-slabs per core for slg/slu
    dsl = F // n_cores
    in_maps = []
    for c in range(n_cores):
        slg = wg_r[:, :, c * nsl:(c + 1) * nsl].reshape(n_layers, -1, 128, 1024)
        slu = wu_r[:, :, c * nsl:(c + 1) * nsl].reshape(n_layers, -1, 128, 1024)
        sld = wd[:, c * dsl:(c + 1) * dsl, :].reshape(n_layers, -1, 128, 1024)
        in_maps.append({
            "x": x[c],
            "rs": rs,
            "wg": wg_r,
            "wu": wu_r,
            "wd": wd,
            "slg": np.ascontiguousarray(slg),
            "slu": np.ascontiguousarray(slu),
            "sld": np.ascontiguousarray(sld),
        })
    return in_maps


def kernel(x, rms_scale, W_g, W_u, W_d):
    """Full-input entry point: shard over batch, run 8-core SPMD, gather."""
    x = np.ascontiguousarray(np.asarray(x, dtype=np.float32))
    rs = np.ascontiguousarray(np.asarray(rms_scale, dtype=np.float32))
    wg = np.ascontiguousarray(np.asarray(W_g, dtype=np.float32))
    wu = np.ascontiguousarray(np.asarray(W_u, dtype=np.float32))
    wd = np.ascontiguousarray(np.asarray(W_d, dtype=np.float32))
    B, S, Dx = x.shape
    assert (B, S, Dx) == (NCORES, TOK, D), (B, S, Dx)
    is_ones = bool(np.all(rs == 1.0))
    nc = _get_nc((is_ones,))
    in_maps = _make_in_maps(x, rs, wg, wu, wd)
    res = run_bass_kernel_spmd(nc, in_maps, list(range(NCORES)))
    return np.stack([res.results[c]["out"] for c in range(NCORES)], axis=0)
